# revision 40
# baseline (speedup 1.0000x reference)
"""Trainium2 Bass kernel for nn_AttentionFusion (dense transformer block).

Sharding: data-parallel over batch. B=8 -> 8 NeuronCores, one element per
core, no collectives. Per core:

  clip (1024ch,16,16) --1x1conv--> (768,16,16) --bilinear 2x--> (768,32,32)
  --channelLN--> q-input;  x -> k, v;  MHA (8 heads, hd=96) -> out-proj -> LN

v4 schedule (PE-bound; keep the tensor engine gapless):
  * q is projected on the 16x16 grid (1x1 conv commutes with bilinear):
    q16 = wq^T c16 costs 1/4 of the 32-grid projection; each head's q16
    is then bilinearly upsampled on the DVE and scaled by rstd.
  * the 32-grid c is never materialized: the channel-LN variance is
    computed exactly from five 16-grid second-moment maps
    (A=E[c^2], B=E[c c_x], C=E[c c_y], D=E[c c_xy], D'=E[c_x c_y])
    pushed through the squared-bilinear stencil.
  * LN1 mean-subtraction folds into the score dot product via a 97th
    feature pair (k97 = x^T (wk @ wqsum), q97 = -mu*rstd), so LN1 never
    touches a 32-grid tensor except the tiny mu/rstd rows.
  * upsamples write x-planar token order (even block | odd block) so all
    blend ops keep packed last dims (DVE 2x); the output DMA un-permutes.
  * DMA order: conv inputs -> wq -> x half -> wk -> x half -> wv -> wo.
  * out-projection runs d-major: oT heads (96 rows) are DMA-repacked
    (SBUF->SBUF) into a [128, 6, S] tile -> 6 accumulating matmuls per
    token tile instead of 8.
  * per-head pipeline: k/q/scores/AV interleaved so softmax exp on ACT
    never blocks the tensor engine; scores are computed [key, query];
    normalization via a ones column in V; no row-max subtracted
    (scores ~N(0,1), exp safe in f32).
"""

import sys
from contextlib import ExitStack

import numpy as np

for _p in ("/opt/trn_rl_repo",):
    if _p not in sys.path:
        sys.path.insert(0, _p)

import concourse.bacc as bacc
import concourse.bass as bass
import concourse.tile as tile
from concourse import mybir
from concourse.bass_utils import run_bass_kernel_spmd

BF16 = mybir.dt.bfloat16
F32 = mybir.dt.float32
F8 = mybir.dt.float8e4
AOP = mybir.AluOpType
AFT = mybir.ActivationFunctionType
DR = mybir.MatmulPerfMode.DoubleRow

B, HH, WW, D = 8, 32, 32, 768
S = HH * WW          # 1024 tokens
CH = 1024            # clip channels
PIX = 256            # 16*16
NH, HD = 8, 96       # heads, head dim
HD1 = HD + 2         # +1 row k97/q97 mean-subtraction feature, +1 zero pad
                     # (DR Ldweights needs NH*HD1 % 16 == 0 -> even width)
P = 128
KT_D = D // P        # 6 contraction tiles over d
KT_C = CH // P       # 8 contraction tiles over clip channels
NT_S = S // P        # 8 token tiles
SCHUNK = 512         # free-dim chunk (one PSUM bank of f32)
NCK = 2              # S // SCHUNK
OCHUNK = 384         # out-proj free chunk (768 = 2*384)
EPS1, EPS2 = 1e-6, 1e-5
SCALE = HD ** -0.5
# second-moment map layout (16-grid): A|B|C|D|D' packed in one staging
# tile; matmul chunks A+B | C+D | D' each fit a 512-f32 PSUM bank
NMAP = 1186          # 256 + 240 + 240 + 225 + 225
MCH = (slice(0, 496), slice(496, 961), slice(961, 1186))
# fp8 weights ship scaled by WS so their residuals clear e4m3's subnormal
# floor (w ~ N(0, 0.036): raw dw ~1e-3 < 2^-9 flushes to zero); the 1/WS
# folds into the PSUM copybacks
WS = 64.0

_TRACE = False
LAST_RESULT = None
_CACHE = {}


def _upsample2x(nc, pool, src, dst, npart, tag, name, bmul):
    """Bilinear 2x upsample [npart,16,16] -> dst [npart,32,32] with
    y-interleaved, x-PLANAR token order (even x block | odd x block), so
    every blend is a TensorTensor with packed last dims (DVE 2x mode).
    half-pixel convention: even: .75*m + .25*(m-1), odd: .75*m + .25*(m+1),
    clamped at edges. `bmul` engine op does the 0.25 scales; corner copies
    on gpsimd."""
    sub = nc.vector.tensor_sub
    b1 = pool.tile([npart, 16, 16], BF16, tag=f"{tag}b1", name=f"{name}_b1")
    bmul(b1, src, 0.25)
    u1 = pool.tile([npart, 16, 16], BF16, tag=f"{tag}u1", name=f"{name}_u1")
    sub(u1, src, b1)
    mid = pool.tile([npart, 16, 32], BF16, tag=f"{tag}mid", name=f"{name}_mid")
    ev = mid[:, :, 0:16]
    od = mid[:, :, 16:32]
    nc.vector.tensor_add(ev[:, :, 1:16], u1[:, :, 1:16], b1[:, :, 0:15])
    nc.gpsimd.tensor_copy(out=ev[:, :, 0:1], in_=src[:, :, 0:1])
    nc.vector.tensor_add(od[:, :, 0:15], u1[:, :, 0:15], b1[:, :, 1:16])
    nc.gpsimd.tensor_copy(out=od[:, :, 15:16], in_=src[:, :, 15:16])
    # y-direction 16 -> 32 (interleaved)
    b2 = pool.tile([npart, 16, 32], BF16, tag=f"{tag}b2", name=f"{name}_b2")
    bmul(b2, mid, 0.25)
    u2 = pool.tile([npart, 16, 32], BF16, tag=f"{tag}u2", name=f"{name}_u2")
    sub(u2, mid, b2)
    dv = dst.rearrange("p (m two) x -> p m two x", two=2)
    dev_ = dv[:, :, 0, :]
    dod = dv[:, :, 1, :]
    nc.vector.tensor_add(dev_[:, 1:16, :], u2[:, 1:16, :], b2[:, 0:15, :])
    nc.gpsimd.tensor_copy(out=dev_[:, 0:1, :], in_=mid[:, 0:1, :])
    nc.vector.tensor_add(dod[:, 0:15, :], u2[:, 0:15, :], b2[:, 1:16, :])
    nc.gpsimd.tensor_copy(out=dod[:, 15:16, :], in_=mid[:, 15:16, :])


def build_graph(flags):
    has_bq, has_bk, has_bv, has_bo, has_ln2 = flags
    nc = bacc.Bacc("TRN2", target_bir_lowering=False)

    # host-prearranged inputs (see make_in_maps for layouts). clip/conv_w/
    # x/wk/wv ship as fp8 hi+lo residual pairs (same bytes as bf16): the
    # matmuls run in DoubleRow perf mode (2 rows/cycle) with a cross-term
    # correction, keeping ~bf16 accuracy at half the PE cost.
    clip_h = nc.dram_tensor("clip", [P, KT_C * 2 * PIX], F8, kind="ExternalInput")
    cw_h = nc.dram_tensor("conv_w", [P, KT_D * KT_C * 2 * P], F8, kind="ExternalInput")
    cb_h = nc.dram_tensor("cb", [P, KT_D], F32, kind="ExternalInput")
    wsum_h = nc.dram_tensor("wsum", [P, KT_C * 2], F8, kind="ExternalInput")
    bsum_h = nc.dram_tensor("bsum", [1, 1], F32, kind="ExternalInput")
    xT_h = nc.dram_tensor("xT", [P, KT_D * 2 * S], F8, kind="ExternalInput")
    wk_h = nc.dram_tensor("wk", [P, KT_D * 2 * NH * HD1], F8, kind="ExternalInput")
    wv_h = nc.dram_tensor("wv", [P, KT_D * 2 * D], F8, kind="ExternalInput")
    wq_h = nc.dram_tensor("wq", [P, KT_D * D], BF16, kind="ExternalInput")
    wo_h = nc.dram_tensor("wo", [P, KT_D * D], BF16, kind="ExternalInput")
    wo6_h = nc.dram_tensor("wo6", [HD, D], BF16, kind="ExternalInput")
    wo7_h = nc.dram_tensor("wo7", [HD, D], BF16, kind="ExternalInput")
    if has_bk:
        bk_h = nc.dram_tensor("bk", [HD1, NH], F32, kind="ExternalInput")
    if has_bq:
        bq_h = nc.dram_tensor("bq", [HD, NH], F32, kind="ExternalInput")
    if has_bv:
        bv_h = nc.dram_tensor("bv", [1, D], F32, kind="ExternalInput")
    if has_bo:
        bo_h = nc.dram_tensor("bo", [1, D], F32, kind="ExternalInput")
    if has_ln2:
        ln2w_h = nc.dram_tensor("ln2w", [1, D], F32, kind="ExternalInput")
        ln2b_h = nc.dram_tensor("ln2b", [1, D], F32, kind="ExternalInput")
    out_h = nc.dram_tensor("out", [S, D], F32, kind="ExternalOutput")
    # rows are written in the x-planar token order; the host un-permutes
    # (free numpy reindex in kernel())

    with tile.TileContext(nc) as tc, ExitStack() as ctx:
        wts = ctx.enter_context(tc.tile_pool(name="wts", bufs=1))
        big = ctx.enter_context(tc.tile_pool(name="big", bufs=1))
        cs = ctx.enter_context(tc.tile_pool(name="cs", bufs=3))
        sq = ctx.enter_context(tc.tile_pool(name="sq", bufs=2))
        qk = ctx.enter_context(tc.tile_pool(name="qk", bufs=4))
        pr = ctx.enter_context(tc.tile_pool(name="pr", bufs=3))
        zp = ctx.enter_context(tc.tile_pool(name="zp", bufs=2))
        op = ctx.enter_context(tc.tile_pool(name="op", bufs=2))
        stp = ctx.enter_context(tc.tile_pool(name="stp", bufs=6))
        # PSUM: psq (k/q proj + st0-pre, <=2KB) x2; scores 4KB x2; psa 2KB x2.
        # Conv-phase psum lives in inner-scope pools freed before pssc/psa.
        psq = ctx.enter_context(tc.tile_pool(name="psq", bufs=2, space="PSUM"))
        # conv-phase-only SBUF (clip, conv weights, LN1-stat rows): freed
        # before oT/oTd/pssc/psa are allocated
        cvw_cm = tc.tile_pool(name="cvw", bufs=1)
        cvw = cvw_cm.__enter__()

        # ---- input DMAs, in compute order, on the sync (SP) queue so the
        # shared DMA device serves them in exactly this order. wq ships
        # before x/wk/wv: q-projection (16-grid) fills the PE gap while the
        # bigger attention inputs stream in. ----
        clip_sb = cvw.tile([P, KT_C, 2, PIX], F8, tag="clip", name="clip_sb")
        clip_r = clip_h[:].rearrange("p (t l x) -> p t l x", t=KT_C, l=2)
        cw_sb = cvw.tile([P, KT_D, KT_C, 2, P], F8, tag="cw", name="cw_sb")
        cw_r = cw_h[:].rearrange("p (t k l m) -> p t k l m", t=KT_D, k=KT_C, l=2)
        # conv_w arranged t-major on host: [P, t, kt, lo/hi, m] so each
        # out-tile's weights arrive in one contiguous DMA; the first conv
        # matmul needs only clip kt0-1 + cw0 kt0-1, so those ship first on
        # two queues (config/HWDGE stages overlap -> conv starts ~2us)
        nc.scalar.dma_start(out=clip_sb[:, 0:2], in_=clip_r[:, 0:2])
        nc.sync.dma_start(out=cw_sb[:, 0, 0:2], in_=cw_r[:, 0, 0:2])
        nc.sync.dma_start(out=cw_sb[:, 0, 2:KT_C], in_=cw_r[:, 0, 2:KT_C])
        nc.sync.dma_start(out=clip_sb[:, 2:KT_C], in_=clip_r[:, 2:KT_C])
        cb_sb = wts.tile([P, KT_D], F32, tag="cb", name="cb_sb")
        nc.gpsimd.dma_start(out=cb_sb, in_=cb_h[:])
        for t in range(1, KT_D):
            nc.sync.dma_start(out=cw_sb[:, t], in_=cw_r[:, t])
        wsum_sb = cvw.tile([P, KT_C, 2, 1], F8, tag="wsum", name="wsum_sb")
        nc.gpsimd.dma_start(
            out=wsum_sb.rearrange("p t l o -> p t (l o)"),
            in_=wsum_h[:].rearrange("p (t l) -> p t l", t=KT_C))
        bsum_sb = cvw.tile([1, 1], F32, tag="bsum", name="bsum_sb")
        nc.gpsimd.dma_start(out=bsum_sb, in_=bsum_h[:])
        wq_sb = wts.tile([P, KT_D, D], BF16, tag="wq", name="wq_sb")
        nc.sync.dma_start(out=wq_sb, in_=wq_h[:].rearrange("p (t d) -> p t d", t=KT_D))
        # x streams in token halves so the first k-projections start early
        xT_sb = wts.tile([P, KT_D, 2, S], F8, tag="xT", name="xT_sb")
        xT_r = xT_h[:].rearrange("p (t l s) -> p t l s", t=KT_D, l=2)
        nc.sync.dma_start(out=xT_sb[:, :, :, 0:SCHUNK], in_=xT_r[:, :, :, 0:SCHUNK])
        wk_sb = wts.tile([P, KT_D, 2, NH * HD1], F8, tag="wk", name="wk_sb")
        nc.sync.dma_start(out=wk_sb, in_=wk_h[:].rearrange("p (t l d) -> p t l d", t=KT_D, l=2))
        nc.sync.dma_start(out=xT_sb[:, :, :, SCHUNK:S], in_=xT_r[:, :, :, SCHUNK:S])
        wv_sb = wts.tile([P, KT_D, 2, D], F8, tag="wv", name="wv_sb")
        nc.sync.dma_start(out=wv_sb, in_=wv_h[:].rearrange("p (t l d) -> p t l d", t=KT_D, l=2))
        wo_sb = wts.tile([P, KT_D, D], BF16, tag="wo", name="wo_sb")
        nc.sync.dma_start(out=wo_sb, in_=wo_h[:].rearrange("p (t d) -> p t d", t=KT_D))
        wo6_sb = wts.tile([HD, D], BF16, tag="wo6", name="wo6_sb")
        nc.sync.dma_start(out=wo6_sb, in_=wo6_h[:])
        wo7_sb = wts.tile([HD, D], BF16, tag="wo7", name="wo7_sb")
        nc.sync.dma_start(out=wo7_sb, in_=wo7_h[:])
        if has_bk:
            bk_sb = wts.tile([HD1, NH], F32, tag="bkk", name="bk_sb")
            nc.sync.dma_start(out=bk_sb, in_=bk_h[:])
        if has_bq:
            bq_sb = wts.tile([HD, NH], F32, tag="bqq", name="bq_sb")
            nc.sync.dma_start(out=bq_sb, in_=bq_h[:])

        ones_bf = wts.tile([P, 1], BF16, tag="onesb", name="ones_bf")
        nc.vector.memset(ones_bf, 1.0)
        eps1_col = wts.tile([1, 1], F32, tag="eps1", name="eps1_col")
        nc.vector.memset(eps1_col, EPS1)
        eps2_col = wts.tile([P, 1], F32, tag="eps2", name="eps2_col")
        nc.vector.memset(eps2_col, EPS2)
        # dummy sqrt so the initial ACT table is sqrt_and_friends (which also
        # covers the conv phase's Identity/Copy/Square) — keeps the var-sqrt
        # off the critical path and leaves just two table switches around exp
        warm = wts.tile([1, 1], F32, tag="warm", name="warm")
        nc.scalar.activation(warm, eps1_col, AFT.Sqrt)

        # ---- persistent activations ----
        c16_sb = big.tile([P, KT_D, PIX], BF16, tag="c16", name="c16_sb")
        v_hsb = big.tile([P, NT_S, NH, HD + 1], BF16, tag="vh", name="v_hsb")
        rstd_b = big.tile([P, S], BF16, tag="rstdb", name="rstd_b")
        nmr_row = big.tile([1, S], BF16, tag="nmr", name="nmr_row")
        nmr2 = big.tile([2, S], BF16, tag="nmr2", name="nmr2")
        if has_bv:
            bv_b = big.tile([P, D], F32, tag="bvb", name="bv_b")
            bv_r = wts.tile([1, D], F32, tag="bvr", name="bv_r")
            nc.sync.dma_start(out=bv_r, in_=bv_h[:])
            nc.gpsimd.partition_broadcast(bv_b, bv_r)
        if has_bo:
            bo_b = big.tile([P, D], F32, tag="bob", name="bo_b")
            bo_r = wts.tile([1, D], F32, tag="bor", name="bo_r")
            nc.sync.dma_start(out=bo_r, in_=bo_h[:])
            nc.gpsimd.partition_broadcast(bo_b, bo_r)
        if has_ln2:
            ln2w_b = big.tile([P, D], F32, tag="l2wb", name="ln2w_b")
            ln2w_r = wts.tile([1, D], F32, tag="l2wr", name="ln2w_r")
            nc.sync.dma_start(out=ln2w_r, in_=ln2w_h[:])
            nc.gpsimd.partition_broadcast(ln2w_b, ln2w_r)
            ln2b_b = big.tile([P, D], F32, tag="l2bb", name="ln2b_b")
            ln2b_r = wts.tile([1, D], F32, tag="l2br", name="ln2b_r")
            nc.sync.dma_start(out=ln2b_r, in_=ln2b_h[:])
            nc.gpsimd.partition_broadcast(ln2b_b, ln2b_r)

        nc.vector.memset(v_hsb[:, :, :, HD:HD + 1], 1.0)

        qs = {}

        q16s = {}

        def q_early(h):
            # 16-grid projection (conv commutes with bilinear); no LN
            # dependency, so this can interleave with the conv-stat matmuls
            hsl = slice(h * HD, (h + 1) * HD)
            pq = psq.tile([HD, PIX], F32, tag="ps", name=f"pq{h}")
            for kt in range(KT_D):
                nc.tensor.matmul(
                    pq, lhsT=wq_sb[:, kt, hsl], rhs=c16_sb[:, kt, :],
                    start=(kt == 0), stop=(kt == KT_D - 1),
                )
            q16 = cs.tile([HD, 16, 16], BF16, tag="q16", name=f"q16_{h}")
            nc.scalar.copy(out=q16, in_=pq.rearrange("p (y x) -> p y x", y=16))
            q16s[h] = q16

        def q_late(h):
            # per-head upsample + LN fold: q = up(q16)*rstd, q97 = -mu*rstd.
            # MUST be emitted after rstd_b/nmr2 are written (program-order
            # read-after-write).
            qs[h] = qk.tile([HD1, S], BF16, tag="q", name=f"q{h}")
            dst = qs[h]
            _upsample2x(nc, cs, q16s[h],
                        dst[0:HD, :].rearrange("p (y x) -> p y x", y=32),
                        HD, "qh", f"q{h}u", nc.gpsimd.tensor_scalar_mul)
            nc.vector.tensor_mul(dst[0:HD, :], dst[0:HD, :], rstd_b[0:HD, :])
            if has_bq:
                nc.vector.tensor_scalar(
                    out=dst[0:HD, :], in0=dst[0:HD, :],
                    scalar1=bq_sb[:, h:h + 1], scalar2=1.0,
                    op0=AOP.add, op1=AOP.mult)
            nc.vector.tensor_copy(out=dst[HD:HD1, :], in_=nmr2)

        def q_head(h):
            q_early(h)
            q_late(h)

        # ================= conv + channel-LN stats phase =================
        m32n = cvw.tile([1, S], BF16, tag="m32n", name="m32n")   # -mu, planar
        var_row = cvw.tile([1, S], BF16, tag="vrow", name="var_row")
        with tc.tile_pool(name="pcv", bufs=2, space="PSUM") as pcv, \
             tc.tile_pool(name="pstat", bufs=1, space="PSUM") as pstat:
            # second-moment sums: 3 bank-sized accumulation chunks
            mm_ps = pstat.tile([1, 3, SCHUNK], F32, tag="mmps", name="mm_ps")

            def conv_tile(t):
                pc = pcv.tile([P, PIX], F32, tag="ps", name=f"pc{t}")
                # hi x hi over kt pairs, then cross-term corrections
                for kp in range(KT_C // 2):
                    nc.tensor.matmul(
                        pc,
                        lhsT=cw_sb[:, t, 2 * kp:2 * kp + 2, 1, :],
                        rhs=clip_sb[:, 2 * kp:2 * kp + 2, 0, :],
                        start=(kp == 0), stop=False, perf_mode=DR,
                    )
                for kt in range(KT_C):
                    nc.tensor.matmul(
                        pc,
                        lhsT=cw_sb[:, t, kt, 0:2, :],
                        rhs=clip_sb[:, kt, 0:2, :],
                        start=False, stop=(kt == KT_C - 1), perf_mode=DR,
                    )
                nc.scalar.activation(
                    c16_sb[:, t].rearrange("p (y x) -> p y x", y=16),
                    pc.rearrange("p (y x) -> p y x", y=16), AFT.Identity,
                    bias=cb_sb[:, t:t + 1], scale=1.0 / WS)
                # 16-grid second-moment products: A|B|C|D|D' packed
                c = c16_sb[:, t].rearrange("p (y x) -> p y x", y=16)
                pd = sq.tile([P, NMAP], BF16, tag="prods", name=f"prods{t}")
                nc.vector.tensor_mul(pd[:, 0:256], c16_sb[:, t], c16_sb[:, t])
                nc.vector.tensor_mul(
                    pd[:, 256:496].rearrange("p (y x) -> p y x", x=15),
                    c[:, :, 0:15], c[:, :, 1:16])
                nc.vector.tensor_mul(pd[:, 496:736], c16_sb[:, t, 0:240],
                                     c16_sb[:, t, 16:256])
                nc.vector.tensor_mul(
                    pd[:, 736:961].rearrange("p (y x) -> p y x", x=15),
                    c[:, 0:15, 0:15], c[:, 1:16, 1:16])
                nc.gpsimd.tensor_mul(
                    out=pd[:, 961:1186].rearrange("p (y x) -> p y x", x=15),
                    in0=c[:, 0:15, 1:16], in1=c[:, 1:16, 0:15])
                prods[t] = pd

            def sq_mm(t):
                # emitted ~2 conv tiles behind prods(t) so the in-order PE
                # never blocks on the conv->maps chain
                for ck in range(3):
                    nc.tensor.matmul(
                        mm_ps[:, ck, 0:MCH[ck].stop - MCH[ck].start],
                        lhsT=ones_bf, rhs=prods[t][:, MCH[ck]],
                        start=(t == 0), stop=(t == KT_D - 1),
                    )

            prods = {}
            conv_tile(0)
            conv_tile(1)
            # mean on the 16x16 grid via host-folded conv_w row-sums,
            # slotted mid-conv right after wsum's DMA lands (fp8 hi parts
            # only: the residual's mean contribution is ~0.07% of c).
            # NEGATED here so m32n = -mu feeds q97 and mu^2 directly.
            mps = mm_ps[0:1, 2, PIX:2 * PIX]   # spare space in bank 2
            for kt in range(KT_C):
                nc.tensor.matmul(
                    mps, lhsT=wsum_sb[:, kt, 0, :], rhs=clip_sb[:, kt, 0, :],
                    start=(kt == 0), stop=(kt == KT_C - 1),
                    skip_group_check=True,
                )
            m16 = cvw.tile([1, 16, 16], BF16, tag="m16", name="m16")
            nc.vector.tensor_scalar(
                out=m16, in0=mps.rearrange("p (y x) -> p y x", y=16),
                scalar1=bsum_sb[0:1, :], scalar2=-1.0 / D,
                op0=AOP.add, op1=AOP.mult,
            )
            _upsample2x(nc, cvw, m16, m32n.rearrange("p (y x) -> p y x", y=32),
                        1, "m", "m", nc.gpsimd.tensor_scalar_mul)
            mu2 = cvw.tile([1, S], BF16, tag="mu2", name="mu2")
            nc.vector.tensor_mul(mu2, m32n, m32n)
            for t in range(2, KT_D):
                conv_tile(t)
                sq_mm(t - 2)
            q_early(0)
            sq_mm(KT_D - 2)
            q_early(1)
            sq_mm(KT_D - 1)
            q_early(2)
            q_early(3)

            # ---- squared-bilinear variance assembly (integer-coefficient
            # stencils; the /16 per stage folds into the final 1/(D*256)).
            # x-stage: M2x = 9*A_i + A_{i-1} + 6*B_{i-1} (planar-x [16,32])
            # DVE reads at most one PSUM operand per op: stage the summed
            # maps to SBUF first (single ACT copy)
            mm_sb = cvw.tile([1, 3, SCHUNK], BF16, tag="mmsb", name="mm_sb")
            nc.scalar.copy(out=mm_sb[:, 0, :], in_=mm_ps[:, 0, :])
            A = mm_sb[0:1, 0, 0:256].rearrange("p (y x) -> p y x", y=16)
            Bm = mm_sb[0:1, 0, 256:496].rearrange("p (y x) -> p y x", y=16)
            Cm = mm_sb[0:1, 1, 0:240].rearrange("p (y x) -> p y x", y=15)
            Dm = mm_sb[0:1, 1, 240:465].rearrange("p (y x) -> p y x", y=15)
            Dp = mm_sb[0:1, 2, 0:225].rearrange("p (y x) -> p y x", y=15)
            m2x = cvw.tile([1, 16, 32], BF16, tag="m2x", name="m2x")
            xm = cvw.tile([1, 15, 32], BF16, tag="xm", name="xm")
            tvar = cvw.tile([1, 16, 32], BF16, tag="tvar", name="tvar")
            ds = cvw.tile([1, 15, 15], BF16, tag="ds", name="ds")
            # M2x even/odd
            nc.vector.scalar_tensor_tensor(
                out=tvar[:, :, 1:16], in0=A[:, :, 1:16], scalar=9.0,
                in1=A[:, :, 0:15], op0=AOP.mult, op1=AOP.add)
            nc.vector.scalar_tensor_tensor(
                out=m2x[:, :, 1:16], in0=Bm[:, :, 0:15], scalar=6.0,
                in1=tvar[:, :, 1:16], op0=AOP.mult, op1=AOP.add)
            nc.vector.tensor_scalar_mul(m2x[:, :, 0:1], A[:, :, 0:1], 16.0)
            nc.vector.scalar_tensor_tensor(
                out=tvar[:, :, 16:31], in0=A[:, :, 0:15], scalar=9.0,
                in1=A[:, :, 1:16], op0=AOP.mult, op1=AOP.add)
            nc.vector.scalar_tensor_tensor(
                out=m2x[:, :, 16:31], in0=Bm[:, :, 0:15], scalar=6.0,
                in1=tvar[:, :, 16:31], op0=AOP.mult, op1=AOP.add)
            nc.vector.tensor_scalar_mul(m2x[:, :, 31:32], A[:, :, 15:16], 16.0)
            # Xm = 9*C_i + C_{i-1} + 3*(D+D')_{i-1}
            nc.scalar.copy(out=mm_sb[:, 1, :], in_=mm_ps[:, 1, :])
            nc.scalar.copy(out=mm_sb[:, 2, 0:PIX], in_=mm_ps[:, 2, 0:PIX])
            nc.vector.tensor_add(ds, Dm, Dp)
            nc.vector.scalar_tensor_tensor(
                out=tvar[:, 0:15, 1:16], in0=Cm[:, :, 1:16], scalar=9.0,
                in1=Cm[:, :, 0:15], op0=AOP.mult, op1=AOP.add)
            nc.vector.scalar_tensor_tensor(
                out=xm[:, :, 1:16], in0=ds, scalar=3.0,
                in1=tvar[:, 0:15, 1:16], op0=AOP.mult, op1=AOP.add)
            nc.vector.tensor_scalar_mul(xm[:, :, 0:1], Cm[:, :, 0:1], 16.0)
            nc.vector.scalar_tensor_tensor(
                out=tvar[:, 0:15, 16:31], in0=Cm[:, :, 0:15], scalar=9.0,
                in1=Cm[:, :, 1:16], op0=AOP.mult, op1=AOP.add)
            nc.vector.scalar_tensor_tensor(
                out=xm[:, :, 16:31], in0=ds, scalar=3.0,
                in1=tvar[:, 0:15, 16:31], op0=AOP.mult, op1=AOP.add)
            nc.vector.tensor_scalar_mul(xm[:, :, 31:32], Cm[:, :, 15:16], 16.0)
            # y-stage into var_row (interleaved y): M2f = 9*M2x_m + M2x_{m∓1}
            # + 6*Xm; then var = M2f/(D*256) - mu^2 in a second pass
            vr = var_row.rearrange("p (m two x) -> p m two x", two=2, x=32)
            vev = vr[:, :, 0, :]
            vod = vr[:, :, 1, :]
            nc.vector.scalar_tensor_tensor(
                out=tvar[:, 1:16, :], in0=m2x[:, 1:16, :], scalar=9.0,
                in1=m2x[:, 0:15, :], op0=AOP.mult, op1=AOP.add)
            nc.vector.scalar_tensor_tensor(
                out=vev[:, 1:16, :], in0=xm, scalar=6.0,
                in1=tvar[:, 1:16, :], op0=AOP.mult, op1=AOP.add)
            nc.vector.tensor_scalar_mul(vev[:, 0:1, :], m2x[:, 0:1, :], 16.0)
            nc.vector.scalar_tensor_tensor(
                out=tvar[:, 0:15, :], in0=m2x[:, 0:15, :], scalar=9.0,
                in1=m2x[:, 1:16, :], op0=AOP.mult, op1=AOP.add)
            nc.vector.scalar_tensor_tensor(
                out=vod[:, 0:15, :], in0=xm, scalar=6.0,
                in1=tvar[:, 0:15, :], op0=AOP.mult, op1=AOP.add)
            nc.vector.tensor_scalar_mul(vod[:, 15:16, :], m2x[:, 15:16, :], 16.0)
            # var = M2f/(D*256) - mu^2 ; rstd = 1/sqrt(var + eps)
            for ck in range(NCK):
                sl = slice(ck * SCHUNK, (ck + 1) * SCHUNK)
                nc.vector.scalar_tensor_tensor(
                    out=var_row[:, sl], in0=var_row[:, sl],
                    scalar=1.0 / (D * 256.0),
                    in1=mu2[:, sl], op0=AOP.mult, op1=AOP.subtract,
                )
        nc.scalar.activation(var_row, var_row, AFT.Sqrt, bias=eps1_col[0:1, :])
        rstd_bf = cvw.tile([1, S], BF16, tag="rstdbf", name="rstd_bf")
        with nc.allow_low_precision(reason="rstd applied to bf16 q anyway"):
            nc.vector.reciprocal(rstd_bf, var_row)
        nc.gpsimd.partition_broadcast(rstd_b, rstd_bf)
        # q97 row: -mu * rstd (bf16), shared across heads; duplicated into
        # a 2-row tile so each q head fills rows 96..97 with one DVE copy
        # (row 97 pairs with a zero k-column, any finite value works)
        nc.vector.tensor_mul(nmr_row, m32n, rstd_bf)
        nc.gpsimd.partition_broadcast(nmr2, nmr_row)

        cvw_cm.__exit__(None, None, None)
        otp = ctx.enter_context(tc.tile_pool(name="otp", bufs=1))
        oT_sb = otp.tile([HD, NH, S], BF16, tag="oT", name="oT_sb")
        # d-major repack of oT for the 6-deep out-projection
        oTd = otp.tile([P, KT_D, S], BF16, tag="oTd", name="oTd")
        att_cm = tc.tile_pool(name="pssc", bufs=2, space="PSUM")
        pssc = att_cm.__enter__()
        att2_cm = tc.tile_pool(name="psa", bufs=2, space="PSUM")
        psa = att2_cm.__enter__()

        # ================= attention =================
        def k_head(h, dst):
            # fp8 DoubleRow: hi x hi over kt pairs + cross-term corrections;
            # moving free is capped at 512 = 2x256, so 256-wide groups.
            # 98 output features: 96 k rows + k97 = x^T (wk @ wqsum) + 0-pad
            hsl = slice(h * HD1, (h + 1) * HD1)
            for ic in range(NCK):
                pp = psq.tile([HD1, SCHUNK], F32, tag="ps", name=f"pp{h}_{ic}")
                for g in range(2):
                    gsl = slice(ic * SCHUNK + g * 256, ic * SCHUNK + (g + 1) * 256)
                    out = pp[:, g * 256:(g + 1) * 256]
                    for tp in range(KT_D // 2):
                        nc.tensor.matmul(
                            out, lhsT=wk_sb[:, 2 * tp:2 * tp + 2, 1, hsl],
                            rhs=xT_sb[:, 2 * tp:2 * tp + 2, 0, gsl],
                            start=(tp == 0), stop=False, perf_mode=DR,
                        )
                    for kt in range(KT_D):
                        nc.tensor.matmul(
                            out, lhsT=wk_sb[:, kt, 0:2, hsl],
                            rhs=xT_sb[:, kt, 0:2, gsl],
                            start=False, stop=(kt == KT_D - 1), perf_mode=DR,
                        )
                isl = slice(ic * SCHUNK, (ic + 1) * SCHUNK)
                if has_bk:
                    nc.vector.tensor_scalar(
                        out=dst[:, isl], in0=pp, scalar1=1.0 / WS,
                        scalar2=bk_sb[:, h:h + 1], op0=AOP.mult, op1=AOP.add)
                elif h < 4:
                    nc.scalar.mul(dst[:, isl], pp, 1.0 / WS)
                else:
                    nc.vector.tensor_scalar_mul(dst[:, isl], pp, 1.0 / WS)

        def scores_head(h, q_sb, k_sb, probs):
            for jt in range(NT_S):
                ps2 = pssc.tile([P, S], F32, tag="sc", name=f"ps{h}_{jt}")
                for ic in range(NCK):
                    isl = slice(ic * SCHUNK, (ic + 1) * SCHUNK)
                    nc.tensor.matmul(
                        ps2[:, isl], lhsT=k_sb[:, jt * P:(jt + 1) * P],
                        rhs=q_sb[:, isl], start=True, stop=True,
                    )
                nc.scalar.activation(probs[:, jt, :], ps2, AFT.Exp, scale=SCALE)

        def repack(h, ic):
            # oT head h (96 rows) -> d-major oTd rows h*96..h*96+95, which
            # spans at most two 128-row tiles; SBUF->SBUF DMA does the
            # partition remap off the engines. Issued per token-half right
            # after each AV copyback, on rotating queues, so the last
            # head's repack latency is mostly hidden.
            f0 = h * HD
            t0, r0 = f0 // P, f0 % P
            n0 = min(P - r0, HD)
            isl = slice(ic * SCHUNK, (ic + 1) * SCHUNK)
            if h >= NH - 2:
                return  # h6/h7 feed the out-projection head-major, no repack
            q0, q1 = (nc.scalar, nc.gpsimd) if ic == 0 else (nc.sync, nc.scalar)
            q0.dma_start(
                out=oTd[r0:r0 + n0, t0, isl], in_=oT_sb[0:n0, h, isl])
            if n0 < HD:
                q1.dma_start(
                    out=oTd[0:HD - n0, t0 + 1, isl], in_=oT_sb[n0:HD, h, isl])

        def av_head(h, probs):
            for ic in range(NCK):
                isl = slice(ic * SCHUNK, (ic + 1) * SCHUNK)
                po = psa.tile([HD + 1, SCHUNK], F32, tag="acc", name=f"po{h}_{ic}")
                for jt in range(NT_S):
                    nc.tensor.matmul(
                        po, lhsT=v_hsb[:, jt, h, :], rhs=probs[:, jt, isl],
                        start=(jt == 0), stop=(jt == NT_S - 1),
                    )
                zr = zp.tile([1, SCHUNK], F32, tag="zr", name=f"zr{h}_{ic}")
                nc.vector.reciprocal(zr, po[HD:HD + 1, :])
                zb = zp.tile([HD, SCHUNK], F32, tag="zb", name=f"zb{h}_{ic}")
                nc.gpsimd.partition_broadcast(zb, zr)
                nc.vector.tensor_mul(oT_sb[:, h, isl], po[0:HD, :], zb)
                repack(h, ic)

        def v_pair(h):
            # V for heads h, h+1 in token-partition layout, 2 jt per PSUM tile
            hsl2 = slice(h * HD, (h + 2) * HD)
            for g in range(4):
                pv = psa.tile([P, 2, 2 * HD], F32, tag="acc", name=f"pv{h}_{g}")
                for j in range(2):
                    jt = 2 * g + j
                    jsl = slice(jt * P, (jt + 1) * P)
                    for tp in range(KT_D // 2):
                        nc.tensor.matmul(
                            pv[:, j, :],
                            lhsT=xT_sb[:, 2 * tp:2 * tp + 2, 0, jsl],
                            rhs=wv_sb[:, 2 * tp:2 * tp + 2, 1, hsl2],
                            start=(tp == 0), stop=False, perf_mode=DR,
                        )
                    for kt in range(KT_D):
                        nc.tensor.matmul(
                            pv[:, j, :],
                            lhsT=xT_sb[:, kt, 0:2, jsl],
                            rhs=wv_sb[:, kt, 0:2, hsl2],
                            start=False, stop=(kt == KT_D - 1), perf_mode=DR,
                        )
                dst = v_hsb[:, 2 * g:2 * g + 2, h:h + 2, 0:HD]
                pv_r = pv.rearrange("p j (hh q) -> p j hh q", hh=2)
                if has_bv:
                    bv_s = bv_b[:, h * HD:(h + 2) * HD].rearrange(
                        "p (hh q) -> p hh q", hh=2)
                    for j in range(2):
                        nc.vector.scalar_tensor_tensor(
                            out=dst[:, j], in0=pv_r[:, j], scalar=1.0 / WS,
                            in1=bv_s, op0=AOP.mult, op1=AOP.add)
                else:
                    nc.vector.tensor_scalar_mul(dst, pv_r, 1.0 / WS)

        ks = {}
        probs = {}

        def k_(h):
            ks[h] = qk.tile([HD1, S], BF16, tag="k", name=f"k{h}")
            k_head(h, ks[h])

        def sc(h):
            probs[h] = pr.tile([P, NT_S, S], BF16, tag="probs", name=f"probs{h}")
            scores_head(h, qs[h], ks[h], probs[h])

        def av(h):
            av_head(h, probs[h])

        # software pipeline: conv (above) -> q heads (16-grid, cheap) while
        # x/wk stream -> k/scores/AV interleaved; scores run ~2 ahead of AV
        # so softmax exp on ACT never blocks the tensor engine.
        q_late(0); q_late(1); q_late(2); q_late(3)
        k_(0); sc(0); k_(1); sc(1); q_head(4)
        k_(2); sc(2); v_pair(0); av(0); q_head(5)
        k_(3); sc(3); av(1); v_pair(2); q_head(6)
        k_(4); sc(4); av(2); q_head(7)
        k_(5); sc(5); av(3); v_pair(4)
        k_(6); sc(6); av(4)
        k_(7); sc(7)
        warm2 = wts.tile([1, 1], F32, tag="warm2", name="warm2")
        nc.scalar.activation(warm2, eps1_col, AFT.Sqrt)
        v_pair(6); av(5)
        # out-proj st0 kt 0..3 accumulate (from the now-idle psq pool) while
        # ACT drains exp(6/7); kt4 joins after av(6)'s repack, kt5 after
        # av(7)'s. (kt tile t is fed by heads floor(t*128/96)..)
        p2s = {}

        def op_mm(p2, st, nk, kt, start, stop, sgc=False):
            # kt 0..3 d-major; kt4 holds only h5's 64 rows; kt5 is replaced
            # by head-major h6+h7 terms read straight from oT_sb
            ssl = slice(st * P, (st + 1) * P)
            nsl = slice(nk * OCHUNK, (nk + 1) * OCHUNK)
            if kt == KT_D - 2:
                nc.tensor.matmul(
                    p2, lhsT=oTd[0:64, kt, ssl], rhs=wo_sb[0:64, kt, nsl],
                    start=start, stop=False, skip_group_check=sgc)
                nc.tensor.matmul(
                    p2, lhsT=oT_sb[:, NH - 2, ssl], rhs=wo6_sb[:, nsl],
                    start=False, stop=stop, skip_group_check=sgc)
            elif kt == KT_D - 1:
                nc.tensor.matmul(
                    p2, lhsT=oT_sb[:, NH - 1, ssl], rhs=wo7_sb[:, nsl],
                    start=start, stop=stop, skip_group_check=sgc)
            else:
                nc.tensor.matmul(
                    p2, lhsT=oTd[:, kt, ssl], rhs=wo_sb[:, kt, nsl],
                    start=start, stop=stop, skip_group_check=sgc)

        def pre_kts(st, kts):
            for nk in range(2):
                if (st, nk) not in p2s:
                    p2s[(st, nk)] = psq.tile(
                        [P, OCHUNK], F32, tag="ps", name=f"po2_{st}_{nk}")
                for kt in kts:
                    op_mm(p2s[(st, nk)], st, nk, kt,
                          kt == 0, False, sgc=True)

        pre_kts(0, range(4))
        av(6)
        pre_kts(0, [4])
        av(7)
        for nk in range(2):
            op_mm(p2s[(0, nk)], 0, nk, 5, False, True, sgc=True)

        # ================= out-projection + final LayerNorm =================
        # attention PSUM pools closed; out-proj uses a 6-buf pool so three
        # token tiles stay in flight while LN2 reads accumulators from PSUM
        att2_cm.__exit__(None, None, None)
        att_cm.__exit__(None, None, None)
        with tc.tile_pool(name="pso", bufs=6, space="PSUM") as pso:
            for st in range(NT_S):
                srcs = []
                st6 = stp.tile([P, 2, 6], F32, tag="st6", name=f"st6_{st}")
                for nk in range(2):
                    if (st, nk) in p2s:
                        p2 = p2s[(st, nk)]
                    else:
                        p2 = pso.tile([P, OCHUNK], F32, tag="po2", name=f"po2_{st}_{nk}")
                        for kt in range(KT_D):
                            op_mm(p2, st, nk, kt, kt == 0, kt == KT_D - 1)
                    if has_bo:
                        tb = op.tile([P, OCHUNK], F32, tag=f"tb{nk}", name=f"tb{st}_{nk}")
                        nc.vector.tensor_add(tb, p2, bo_b[:, nk * OCHUNK:(nk + 1) * OCHUNK])
                        srcs.append(tb)
                    else:
                        srcs.append(p2)
                    nc.vector.bn_stats(out=st6[:, nk, :], in_=srcs[nk])
                mv = stp.tile([P, 2], F32, tag="mv", name=f"mv{st}")
                nc.vector.bn_aggr(out=mv, in_=st6)
                stdc = stp.tile([P, 1], F32, tag="stdc", name=f"stdc{st}")
                nc.scalar.activation(stdc, mv[:, 1:2], AFT.Sqrt, bias=eps2_col)
                rstdc = stp.tile([P, 1], F32, tag="rstdc", name=f"rstdc{st}")
                nc.vector.reciprocal(rstdc, stdc)
                # normalize on ACT: out = src*rstd + (-mu*rstd), per-partition
                nmr = stp.tile([P, 1], F32, tag="nmr", name=f"nmr{st}")
                nc.vector.tensor_scalar(
                    out=nmr, in0=mv[:, 0:1], scalar1=rstdc, scalar2=-1.0,
                    op0=AOP.mult, op1=AOP.mult,
                )
                out_sb = op.tile([P, D], F32, tag="out", name=f"out_sb{st}")
                nc.scalar.activation(
                    out_sb[:, 0:OCHUNK], srcs[0], AFT.Identity,
                    bias=nmr, scale=rstdc)
                nc.vector.tensor_scalar(
                    out=out_sb[:, OCHUNK:D], in0=srcs[1], scalar1=mv[:, 0:1],
                    scalar2=rstdc, op0=AOP.subtract, op1=AOP.mult)
                if has_ln2:
                    nc.vector.tensor_mul(out_sb, out_sb, ln2w_b)
                    nc.vector.tensor_add(out_sb, out_sb, ln2b_b)
                orow = out_h[:][st * P:(st + 1) * P, :]
                if st == NT_S - 1:
                    for c0, qe in ((0, nc.sync), (192, nc.scalar),
                                   (384, nc.gpsimd), (576, nc.sync)):
                        qe.dma_start(out=orow[:, c0:c0 + 192],
                                     in_=out_sb[:, c0:c0 + 192])
                else:
                    nc.sync.dma_start(out=orow[:, 0:OCHUNK], in_=out_sb[:, 0:OCHUNK])
                    nc.scalar.dma_start(out=orow[:, OCHUNK:D], in_=out_sb[:, OCHUNK:D])

    nc.compile()
    return nc


def _get_graph(flags):
    if flags not in _CACHE:
        _CACHE[flags] = build_graph(flags)
    return _CACHE[flags]


def make_in_maps(**inputs):
    """Host-side prep: fold ln1 into wq, cast to bf16, transpose x."""
    import ml_dtypes

    bf = ml_dtypes.bfloat16
    f32 = np.float32
    x = np.asarray(inputs["x"], f32)
    clip = np.asarray(inputs["clip_features"], f32)
    conv_w = np.asarray(inputs["conv_w"], f32)
    conv_b = np.asarray(inputs["conv_b"], f32)
    ln1_w = np.asarray(inputs["ln1_w"], f32)
    ln1_b = np.asarray(inputs["ln1_b"], f32)
    wq = np.asarray(inputs["wq"], f32)
    bq = np.asarray(inputs["bq"], f32)
    wk = np.asarray(inputs["wk"], f32)
    bk = np.asarray(inputs["bk"], f32)
    wv = np.asarray(inputs["wv"], f32)
    bv = np.asarray(inputs["bv"], f32)
    wo = np.asarray(inputs["wo"], f32)
    bo = np.asarray(inputs["bo"], f32)
    ln2_w = np.asarray(inputs["ln2_w"], f32)
    ln2_b = np.asarray(inputs["ln2_b"], f32)

    wq_eff = ln1_w[:, None] * wq
    bq_eff = bq + ln1_b @ wq
    # q97/k97 mean-subtraction feature: wqsum[m] = sum_d wq_eff[d, m];
    # per head h, wk97_h = wk_h @ wqsum_h so that
    # k97 = x^T wk97 pairs with q97 = -mu*rstd in the score dot product.
    wqsum = wq_eff.sum(axis=0)                       # [D]
    wk_h = wk.reshape(D, NH, HD)
    ws_h = wqsum.reshape(NH, HD)
    wk97 = np.einsum("dhm,hm->dh", wk_h, ws_h)       # [D, NH]
    wk_ext = np.concatenate(
        [wk_h, wk97[:, :, None], np.zeros((D, NH, 1), f32)],
        axis=2).reshape(D, NH * HD1)

    flags = (
        bool(np.any(bq_eff)),
        bool(np.any(bk)),
        bool(np.any(bv)),
        bool(np.any(bo)),
        bool(np.any(ln2_w != 1.0) or np.any(ln2_b)),
    )

    e4 = ml_dtypes.float8_e4m3

    def hilo(a):  # activations: fp8 hi at index 0, residual lo at 1 (axis -2)
        hi = np.asarray(a, f32).astype(e4)
        lo = (np.asarray(a, f32) - hi.astype(f32)).astype(e4)
        return np.stack([hi, lo], axis=-2)

    def lohi_w(a):  # weights: scaled by WS; residual at 0, main at 1
        s = np.asarray(a, f32) * WS
        hi = s.astype(e4)
        lo = (s - hi.astype(f32)).astype(e4)
        return np.stack([lo, hi], axis=-2)

    def dev_kp(w):  # [K, M] -> [P, kt, M], k-tile-major
        kt = w.shape[0] // P
        return w.reshape(kt, P, w.shape[1]).transpose(1, 0, 2)

    # conv_w t-major: [P, t, kt, {dw,w}, m] with ch = kt*128+p, d = t*128+m
    cw_t = conv_w.reshape(KT_C, P, KT_D, P).transpose(1, 2, 0, 3)
    wsum8 = conv_w.sum(axis=1).reshape(KT_C, P).T.astype(e4)  # [P, kt]

    shared = {
        "conv_w": np.ascontiguousarray(lohi_w(cw_t)).reshape(P, -1),
        "wsum": np.ascontiguousarray(
            np.stack([wsum8, wsum8], axis=-1)).reshape(P, -1),
        "bsum": np.full((1, 1), conv_b.sum(), dtype=f32),
        "wq": np.ascontiguousarray(dev_kp(wq_eff).astype(bf)).reshape(P, -1),
        "wk": np.ascontiguousarray(lohi_w(dev_kp(wk_ext))).reshape(P, -1),
        "wv": np.ascontiguousarray(lohi_w(dev_kp(wv))).reshape(P, -1),
        "wo": np.ascontiguousarray(dev_kp(wo).astype(bf)).reshape(P, -1),
        "wo6": np.ascontiguousarray(wo[(NH - 2) * HD:(NH - 1) * HD, :].astype(bf)),
        "wo7": np.ascontiguousarray(wo[(NH - 1) * HD:, :].astype(bf)),
        "cb": np.ascontiguousarray(conv_b.reshape(KT_D, P).T, dtype=f32),
    }
    if flags[0]:
        shared["bq"] = np.ascontiguousarray(bq_eff.reshape(NH, HD).T, dtype=f32)
    if flags[1]:
        bk97 = np.einsum("hm,hm->h", bk.reshape(NH, HD), ws_h)
        bk_ext = np.concatenate(
            [bk.reshape(NH, HD), bk97[:, None],
             np.zeros((NH, 1), f32)], axis=1)              # [NH, HD1]
        shared["bk"] = np.ascontiguousarray(bk_ext.T, dtype=f32)
    if flags[2]:
        shared["bv"] = np.ascontiguousarray(bv[None, :], dtype=f32)
    if flags[3]:
        shared["bo"] = np.ascontiguousarray(bo[None, :], dtype=f32)
    if flags[4]:
        shared["ln2w"] = np.ascontiguousarray(ln2_w[None, :], dtype=f32)
        shared["ln2b"] = np.ascontiguousarray(ln2_b[None, :], dtype=f32)

    in_maps = []
    for b in range(B):
        m = dict(shared)
        m["xT"] = np.ascontiguousarray(
            hilo(dev_kp(x[b].reshape(S, D).T))).reshape(P, -1)
        m["clip"] = np.ascontiguousarray(
            hilo(dev_kp(clip[b].reshape(CH, PIX)))).reshape(P, -1)
        in_maps.append(m)
    return flags, in_maps


def kernel(**inputs):
    global LAST_RESULT
    flags, in_maps = make_in_maps(**inputs)
    nc = _get_graph(flags)
    res = run_bass_kernel_spmd(nc, in_maps, core_ids=list(range(B)), trace=_TRACE)
    LAST_RESULT = res
    out = np.stack([r["out"] for r in res.results], axis=0)
    # un-permute x-planar token order: dram row rho holds raster token
    # (rho//32)*32 + 2*(rho%16) + (rho%32)//16
    rho = np.arange(S)
    raster = (rho // 32) * 32 + 2 * (rho % 16) + (rho % 32) // 16
    inv = np.empty(S, np.int64)
    inv[raster] = rho
    out = out[:, inv, :]
    return np.ascontiguousarray(out.reshape(B, HH, WW, D), dtype=np.float32)


# revision 41
# speedup vs baseline: 1.0024x; 1.0024x over previous
"""Trainium2 Bass kernel for nn_AttentionFusion (dense transformer block).

Sharding: data-parallel over batch. B=8 -> 8 NeuronCores, one element per
core, no collectives. Per core:

  clip (1024ch,16,16) --1x1conv--> (768,16,16) --bilinear 2x--> (768,32,32)
  --channelLN--> q-input;  x -> k, v;  MHA (8 heads, hd=96) -> out-proj -> LN

v4 schedule (PE-bound; keep the tensor engine gapless):
  * q is projected on the 16x16 grid (1x1 conv commutes with bilinear):
    q16 = wq^T c16 costs 1/4 of the 32-grid projection; each head's q16
    is then bilinearly upsampled on the DVE and scaled by rstd.
  * the 32-grid c is never materialized: the channel-LN variance is
    computed exactly from five 16-grid second-moment maps
    (A=E[c^2], B=E[c c_x], C=E[c c_y], D=E[c c_xy], D'=E[c_x c_y])
    pushed through the squared-bilinear stencil.
  * LN1 mean-subtraction folds into the score dot product via a 97th
    feature pair (k97 = x^T (wk @ wqsum), q97 = -mu*rstd), so LN1 never
    touches a 32-grid tensor except the tiny mu/rstd rows.
  * upsamples write x-planar token order (even block | odd block) so all
    blend ops keep packed last dims (DVE 2x); the output DMA un-permutes.
  * DMA order: conv inputs -> wq -> x half -> wk -> x half -> wv -> wo.
  * out-projection runs d-major: oT heads (96 rows) are DMA-repacked
    (SBUF->SBUF) into a [128, 6, S] tile -> 6 accumulating matmuls per
    token tile instead of 8.
  * per-head pipeline: k/q/scores/AV interleaved so softmax exp on ACT
    never blocks the tensor engine; scores are computed [key, query];
    normalization via a ones column in V; no row-max subtracted
    (scores ~N(0,1), exp safe in f32).
"""

import sys
from contextlib import ExitStack

import numpy as np

for _p in ("/opt/trn_rl_repo",):
    if _p not in sys.path:
        sys.path.insert(0, _p)

import concourse.bacc as bacc
import concourse.bass as bass
import concourse.tile as tile
from concourse import mybir
from concourse.bass_utils import run_bass_kernel_spmd

BF16 = mybir.dt.bfloat16
F32 = mybir.dt.float32
F8 = mybir.dt.float8e4
AOP = mybir.AluOpType
AFT = mybir.ActivationFunctionType
DR = mybir.MatmulPerfMode.DoubleRow

B, HH, WW, D = 8, 32, 32, 768
S = HH * WW          # 1024 tokens
CH = 1024            # clip channels
PIX = 256            # 16*16
NH, HD = 8, 96       # heads, head dim
HD1 = HD + 2         # +1 row k97/q97 mean-subtraction feature, +1 zero pad
                     # (DR Ldweights needs NH*HD1 % 16 == 0 -> even width)
P = 128
KT_D = D // P        # 6 contraction tiles over d
KT_C = CH // P       # 8 contraction tiles over clip channels
NT_S = S // P        # 8 token tiles
SCHUNK = 512         # free-dim chunk (one PSUM bank of f32)
NCK = 2              # S // SCHUNK
OCHUNK = 384         # out-proj free chunk (768 = 2*384)
EPS1, EPS2 = 1e-6, 1e-5
SCALE = HD ** -0.5
# second-moment map layout (16-grid): A|B|C|D|D' packed in one staging
# tile; matmul chunks A+B | C+D | D' each fit a 512-f32 PSUM bank
NMAP = 1186          # 256 + 240 + 240 + 225 + 225
MCH = (slice(0, 496), slice(496, 961), slice(961, 1186))
# fp8 weights ship scaled by WS so their residuals clear e4m3's subnormal
# floor (w ~ N(0, 0.036): raw dw ~1e-3 < 2^-9 flushes to zero); the 1/WS
# folds into the PSUM copybacks
WS = 64.0

_TRACE = False
LAST_RESULT = None
_CACHE = {}


def _upsample2x(nc, pool, src, dst, npart, tag, name, bmul):
    """Bilinear 2x upsample [npart,16,16] -> dst [npart,32,32] with
    y-interleaved, x-PLANAR token order (even x block | odd x block), so
    every blend is a TensorTensor with packed last dims (DVE 2x mode).
    half-pixel convention: even: .75*m + .25*(m-1), odd: .75*m + .25*(m+1),
    clamped at edges. `bmul` engine op does the 0.25 scales; corner copies
    on gpsimd."""
    sub = nc.vector.tensor_sub
    b1 = pool.tile([npart, 16, 16], BF16, tag=f"{tag}b1", name=f"{name}_b1")
    bmul(b1, src, 0.25)
    u1 = pool.tile([npart, 16, 16], BF16, tag=f"{tag}u1", name=f"{name}_u1")
    sub(u1, src, b1)
    mid = pool.tile([npart, 16, 32], BF16, tag=f"{tag}mid", name=f"{name}_mid")
    ev = mid[:, :, 0:16]
    od = mid[:, :, 16:32]
    nc.vector.tensor_add(ev[:, :, 1:16], u1[:, :, 1:16], b1[:, :, 0:15])
    nc.gpsimd.tensor_copy(out=ev[:, :, 0:1], in_=src[:, :, 0:1])
    nc.vector.tensor_add(od[:, :, 0:15], u1[:, :, 0:15], b1[:, :, 1:16])
    nc.gpsimd.tensor_copy(out=od[:, :, 15:16], in_=src[:, :, 15:16])
    # y-direction 16 -> 32 (interleaved)
    b2 = pool.tile([npart, 16, 32], BF16, tag=f"{tag}b2", name=f"{name}_b2")
    bmul(b2, mid, 0.25)
    u2 = pool.tile([npart, 16, 32], BF16, tag=f"{tag}u2", name=f"{name}_u2")
    sub(u2, mid, b2)
    dv = dst.rearrange("p (m two) x -> p m two x", two=2)
    dev_ = dv[:, :, 0, :]
    dod = dv[:, :, 1, :]
    nc.vector.tensor_add(dev_[:, 1:16, :], u2[:, 1:16, :], b2[:, 0:15, :])
    nc.gpsimd.tensor_copy(out=dev_[:, 0:1, :], in_=mid[:, 0:1, :])
    nc.vector.tensor_add(dod[:, 0:15, :], u2[:, 0:15, :], b2[:, 1:16, :])
    nc.gpsimd.tensor_copy(out=dod[:, 15:16, :], in_=mid[:, 15:16, :])


def build_graph(flags):
    has_bq, has_bk, has_bv, has_bo, has_ln2 = flags
    nc = bacc.Bacc("TRN2", target_bir_lowering=False)

    # host-prearranged inputs (see make_in_maps for layouts). clip/conv_w/
    # x/wk/wv ship as fp8 hi+lo residual pairs (same bytes as bf16): the
    # matmuls run in DoubleRow perf mode (2 rows/cycle) with a cross-term
    # correction, keeping ~bf16 accuracy at half the PE cost.
    clip_h = nc.dram_tensor("clip", [P, KT_C * 2 * PIX], F8, kind="ExternalInput")
    cw_h = nc.dram_tensor("conv_w", [P, KT_D * KT_C * 2 * P], F8, kind="ExternalInput")
    cb_h = nc.dram_tensor("cb", [P, KT_D], F32, kind="ExternalInput")
    wsum_h = nc.dram_tensor("wsum", [P, KT_C * 2], F8, kind="ExternalInput")
    bsum_h = nc.dram_tensor("bsum", [1, 1], F32, kind="ExternalInput")
    xT_h = nc.dram_tensor("xT", [P, KT_D * 2 * S], F8, kind="ExternalInput")
    wk_h = nc.dram_tensor("wk", [P, KT_D * 2 * NH * HD1], F8, kind="ExternalInput")
    wv_h = nc.dram_tensor("wv", [P, KT_D * 2 * D], F8, kind="ExternalInput")
    wq_h = nc.dram_tensor("wq", [P, KT_D * D], BF16, kind="ExternalInput")
    wo_h = nc.dram_tensor("wo", [P, KT_D * D], BF16, kind="ExternalInput")
    wo6_h = nc.dram_tensor("wo6", [HD, D], BF16, kind="ExternalInput")
    wo7_h = nc.dram_tensor("wo7", [HD, D], BF16, kind="ExternalInput")
    if has_bk:
        bk_h = nc.dram_tensor("bk", [HD1, NH], F32, kind="ExternalInput")
    if has_bq:
        bq_h = nc.dram_tensor("bq", [HD, NH], F32, kind="ExternalInput")
    if has_bv:
        bv_h = nc.dram_tensor("bv", [1, D], F32, kind="ExternalInput")
    if has_bo:
        bo_h = nc.dram_tensor("bo", [1, D], F32, kind="ExternalInput")
    if has_ln2:
        ln2w_h = nc.dram_tensor("ln2w", [1, D], F32, kind="ExternalInput")
        ln2b_h = nc.dram_tensor("ln2b", [1, D], F32, kind="ExternalInput")
    out_h = nc.dram_tensor("out", [S, D], F32, kind="ExternalOutput")
    # rows are written in the x-planar token order; the host un-permutes
    # (free numpy reindex in kernel())

    with tile.TileContext(nc) as tc, ExitStack() as ctx:
        wts = ctx.enter_context(tc.tile_pool(name="wts", bufs=1))
        big = ctx.enter_context(tc.tile_pool(name="big", bufs=1))
        cs = ctx.enter_context(tc.tile_pool(name="cs", bufs=3))
        sq = ctx.enter_context(tc.tile_pool(name="sq", bufs=2))
        qk = ctx.enter_context(tc.tile_pool(name="qk", bufs=4))
        pr = ctx.enter_context(tc.tile_pool(name="pr", bufs=3))
        zp = ctx.enter_context(tc.tile_pool(name="zp", bufs=2))
        op = ctx.enter_context(tc.tile_pool(name="op", bufs=2))
        stp = ctx.enter_context(tc.tile_pool(name="stp", bufs=6))
        # PSUM: psq (k/q proj + st0-pre, <=2KB) x2; scores 4KB x2; psa 2KB x2.
        # Conv-phase psum lives in inner-scope pools freed before pssc/psa.
        psq = ctx.enter_context(tc.tile_pool(name="psq", bufs=2, space="PSUM"))
        # conv-phase-only SBUF (clip, conv weights, LN1-stat rows): freed
        # before oT/oTd/pssc/psa are allocated
        cvw_cm = tc.tile_pool(name="cvw", bufs=1)
        cvw = cvw_cm.__enter__()

        # ---- input DMAs, in compute order, on the sync (SP) queue so the
        # shared DMA device serves them in exactly this order. wq ships
        # before x/wk/wv: q-projection (16-grid) fills the PE gap while the
        # bigger attention inputs stream in. ----
        clip_sb = cvw.tile([P, KT_C, 2, PIX], F8, tag="clip", name="clip_sb")
        clip_r = clip_h[:].rearrange("p (t l x) -> p t l x", t=KT_C, l=2)
        cw_sb = cvw.tile([P, KT_D, KT_C, 2, P], F8, tag="cw", name="cw_sb")
        cw_r = cw_h[:].rearrange("p (t k l m) -> p t k l m", t=KT_D, k=KT_C, l=2)
        # conv_w arranged t-major on host: [P, t, kt, lo/hi, m] so each
        # out-tile's weights arrive in one contiguous DMA; the first conv
        # matmul needs only clip kt0-1 + cw0 kt0-1, so those ship first on
        # two queues (config/HWDGE stages overlap -> conv starts ~2us)
        nc.scalar.dma_start(out=clip_sb[:, 0:2], in_=clip_r[:, 0:2])
        nc.sync.dma_start(out=cw_sb[:, 0, 0:2], in_=cw_r[:, 0, 0:2])
        nc.sync.dma_start(out=cw_sb[:, 0, 2:KT_C], in_=cw_r[:, 0, 2:KT_C])
        nc.sync.dma_start(out=clip_sb[:, 2:KT_C], in_=clip_r[:, 2:KT_C])
        cb_sb = wts.tile([P, KT_D], F32, tag="cb", name="cb_sb")
        nc.gpsimd.dma_start(out=cb_sb, in_=cb_h[:])
        for t in range(1, KT_D):
            nc.sync.dma_start(out=cw_sb[:, t], in_=cw_r[:, t])
        wsum_sb = cvw.tile([P, KT_C, 2, 1], F8, tag="wsum", name="wsum_sb")
        nc.gpsimd.dma_start(
            out=wsum_sb.rearrange("p t l o -> p t (l o)"),
            in_=wsum_h[:].rearrange("p (t l) -> p t l", t=KT_C))
        bsum_sb = cvw.tile([1, 1], F32, tag="bsum", name="bsum_sb")
        nc.gpsimd.dma_start(out=bsum_sb, in_=bsum_h[:])
        wq_sb = wts.tile([P, KT_D, D], BF16, tag="wq", name="wq_sb")
        nc.sync.dma_start(out=wq_sb, in_=wq_h[:].rearrange("p (t d) -> p t d", t=KT_D))
        # x streams in token halves so the first k-projections start early
        xT_sb = wts.tile([P, KT_D, 2, S], F8, tag="xT", name="xT_sb")
        xT_r = xT_h[:].rearrange("p (t l s) -> p t l s", t=KT_D, l=2)
        nc.sync.dma_start(out=xT_sb[:, :, :, 0:SCHUNK], in_=xT_r[:, :, :, 0:SCHUNK])
        wk_sb = wts.tile([P, KT_D, 2, NH * HD1], F8, tag="wk", name="wk_sb")
        nc.sync.dma_start(out=wk_sb, in_=wk_h[:].rearrange("p (t l d) -> p t l d", t=KT_D, l=2))
        nc.sync.dma_start(out=xT_sb[:, :, :, SCHUNK:S], in_=xT_r[:, :, :, SCHUNK:S])
        wv_sb = wts.tile([P, KT_D, 2, D], F8, tag="wv", name="wv_sb")
        nc.sync.dma_start(out=wv_sb, in_=wv_h[:].rearrange("p (t l d) -> p t l d", t=KT_D, l=2))
        wo_sb = wts.tile([P, KT_D, D], BF16, tag="wo", name="wo_sb")
        nc.sync.dma_start(out=wo_sb, in_=wo_h[:].rearrange("p (t d) -> p t d", t=KT_D))
        wo6_sb = wts.tile([HD, D], BF16, tag="wo6", name="wo6_sb")
        nc.sync.dma_start(out=wo6_sb, in_=wo6_h[:])
        wo7_sb = wts.tile([HD, D], BF16, tag="wo7", name="wo7_sb")
        nc.sync.dma_start(out=wo7_sb, in_=wo7_h[:])
        if has_bk:
            bk_sb = wts.tile([HD1, NH], F32, tag="bkk", name="bk_sb")
            nc.sync.dma_start(out=bk_sb, in_=bk_h[:])
        if has_bq:
            bq_sb = wts.tile([HD, NH], F32, tag="bqq", name="bq_sb")
            nc.sync.dma_start(out=bq_sb, in_=bq_h[:])

        ones_bf = wts.tile([P, 1], BF16, tag="onesb", name="ones_bf")
        nc.vector.memset(ones_bf, 1.0)
        eps1_col = wts.tile([1, 1], F32, tag="eps1", name="eps1_col")
        nc.vector.memset(eps1_col, EPS1)
        eps2_col = wts.tile([P, 1], F32, tag="eps2", name="eps2_col")
        nc.vector.memset(eps2_col, EPS2)
        # dummy sqrt so the initial ACT table is sqrt_and_friends (which also
        # covers the conv phase's Identity/Copy/Square) — keeps the var-sqrt
        # off the critical path and leaves just two table switches around exp
        warm = wts.tile([1, 1], F32, tag="warm", name="warm")
        nc.scalar.activation(warm, eps1_col, AFT.Sqrt)

        # ---- persistent activations ----
        c16_sb = big.tile([P, KT_D, PIX], BF16, tag="c16", name="c16_sb")
        v_hsb = big.tile([P, NT_S, NH, HD + 1], BF16, tag="vh", name="v_hsb")
        rstd_b = big.tile([P, S], BF16, tag="rstdb", name="rstd_b")
        nmr_row = big.tile([1, S], BF16, tag="nmr", name="nmr_row")
        nmr2 = big.tile([2, S], BF16, tag="nmr2", name="nmr2")
        if has_bv:
            bv_b = big.tile([P, D], F32, tag="bvb", name="bv_b")
            bv_r = wts.tile([1, D], F32, tag="bvr", name="bv_r")
            nc.sync.dma_start(out=bv_r, in_=bv_h[:])
            nc.gpsimd.partition_broadcast(bv_b, bv_r)
        if has_bo:
            bo_b = big.tile([P, D], F32, tag="bob", name="bo_b")
            bo_r = wts.tile([1, D], F32, tag="bor", name="bo_r")
            nc.sync.dma_start(out=bo_r, in_=bo_h[:])
            nc.gpsimd.partition_broadcast(bo_b, bo_r)
        if has_ln2:
            ln2w_b = big.tile([P, D], F32, tag="l2wb", name="ln2w_b")
            ln2w_r = wts.tile([1, D], F32, tag="l2wr", name="ln2w_r")
            nc.sync.dma_start(out=ln2w_r, in_=ln2w_h[:])
            nc.gpsimd.partition_broadcast(ln2w_b, ln2w_r)
            ln2b_b = big.tile([P, D], F32, tag="l2bb", name="ln2b_b")
            ln2b_r = wts.tile([1, D], F32, tag="l2br", name="ln2b_r")
            nc.sync.dma_start(out=ln2b_r, in_=ln2b_h[:])
            nc.gpsimd.partition_broadcast(ln2b_b, ln2b_r)

        nc.vector.memset(v_hsb[:, :, :, HD:HD + 1], 1.0)

        qs = {}

        q16s = {}

        def q_early(h):
            # 16-grid projection (conv commutes with bilinear); no LN
            # dependency, so this can interleave with the conv-stat matmuls
            hsl = slice(h * HD, (h + 1) * HD)
            pq = psq.tile([HD, PIX], F32, tag="ps", name=f"pq{h}")
            for kt in range(KT_D):
                nc.tensor.matmul(
                    pq, lhsT=wq_sb[:, kt, hsl], rhs=c16_sb[:, kt, :],
                    start=(kt == 0), stop=(kt == KT_D - 1),
                )
            q16 = cs.tile([HD, 16, 16], BF16, tag="q16", name=f"q16_{h}")
            nc.scalar.copy(out=q16, in_=pq.rearrange("p (y x) -> p y x", y=16))
            q16s[h] = q16

        def q_late(h):
            # per-head upsample + LN fold: q = up(q16)*rstd, q97 = -mu*rstd.
            # MUST be emitted after rstd_b/nmr2 are written (program-order
            # read-after-write).
            qs[h] = qk.tile([HD1, S], BF16, tag="q", name=f"q{h}")
            dst = qs[h]
            _upsample2x(nc, cs, q16s[h],
                        dst[0:HD, :].rearrange("p (y x) -> p y x", y=32),
                        HD, "qh", f"q{h}u", nc.gpsimd.tensor_scalar_mul)
            nc.vector.tensor_mul(dst[0:HD, :], dst[0:HD, :], rstd_b[0:HD, :])
            if has_bq:
                nc.vector.tensor_scalar(
                    out=dst[0:HD, :], in0=dst[0:HD, :],
                    scalar1=bq_sb[:, h:h + 1], scalar2=1.0,
                    op0=AOP.add, op1=AOP.mult)
            nc.vector.tensor_copy(out=dst[HD:HD1, :], in_=nmr2)

        def q_head(h):
            q_early(h)
            q_late(h)

        # ================= conv + channel-LN stats phase =================
        m32n = cvw.tile([1, S], BF16, tag="m32n", name="m32n")   # -mu, planar
        var_row = cvw.tile([1, S], BF16, tag="vrow", name="var_row")
        with tc.tile_pool(name="pcv", bufs=2, space="PSUM") as pcv, \
             tc.tile_pool(name="pstat", bufs=1, space="PSUM") as pstat:
            # second-moment sums: 3 bank-sized accumulation chunks
            mm_ps = pstat.tile([1, 3, SCHUNK], F32, tag="mmps", name="mm_ps")

            def conv_tile(t):
                pc = pcv.tile([P, PIX], F32, tag="ps", name=f"pc{t}")
                # hi x hi over kt pairs, then cross-term corrections
                for kp in range(KT_C // 2):
                    nc.tensor.matmul(
                        pc,
                        lhsT=cw_sb[:, t, 2 * kp:2 * kp + 2, 1, :],
                        rhs=clip_sb[:, 2 * kp:2 * kp + 2, 0, :],
                        start=(kp == 0), stop=False, perf_mode=DR,
                    )
                for kt in range(KT_C):
                    nc.tensor.matmul(
                        pc,
                        lhsT=cw_sb[:, t, kt, 0:2, :],
                        rhs=clip_sb[:, kt, 0:2, :],
                        start=False, stop=(kt == KT_C - 1), perf_mode=DR,
                    )
                nc.scalar.activation(
                    c16_sb[:, t].rearrange("p (y x) -> p y x", y=16),
                    pc.rearrange("p (y x) -> p y x", y=16), AFT.Identity,
                    bias=cb_sb[:, t:t + 1], scale=1.0 / WS)
                # 16-grid second-moment products: A|B|C|D|D' packed
                c = c16_sb[:, t].rearrange("p (y x) -> p y x", y=16)
                pd = sq.tile([P, NMAP], BF16, tag="prods", name=f"prods{t}")
                nc.vector.tensor_mul(pd[:, 0:256], c16_sb[:, t], c16_sb[:, t])
                nc.vector.tensor_mul(
                    pd[:, 256:496].rearrange("p (y x) -> p y x", x=15),
                    c[:, :, 0:15], c[:, :, 1:16])
                nc.vector.tensor_mul(pd[:, 496:736], c16_sb[:, t, 0:240],
                                     c16_sb[:, t, 16:256])
                nc.vector.tensor_mul(
                    pd[:, 736:961].rearrange("p (y x) -> p y x", x=15),
                    c[:, 0:15, 0:15], c[:, 1:16, 1:16])
                nc.gpsimd.tensor_mul(
                    out=pd[:, 961:1186].rearrange("p (y x) -> p y x", x=15),
                    in0=c[:, 0:15, 1:16], in1=c[:, 1:16, 0:15])
                prods[t] = pd

            def sq_mm(t):
                # emitted ~2 conv tiles behind prods(t) so the in-order PE
                # never blocks on the conv->maps chain
                for ck in range(3):
                    nc.tensor.matmul(
                        mm_ps[:, ck, 0:MCH[ck].stop - MCH[ck].start],
                        lhsT=ones_bf, rhs=prods[t][:, MCH[ck]],
                        start=(t == 0), stop=(t == KT_D - 1),
                    )

            prods = {}
            conv_tile(0)
            conv_tile(1)
            # mean on the 16x16 grid via host-folded conv_w row-sums,
            # slotted mid-conv right after wsum's DMA lands (fp8 hi parts
            # only: the residual's mean contribution is ~0.07% of c).
            # NEGATED here so m32n = -mu feeds q97 and mu^2 directly.
            mps = mm_ps[0:1, 2, PIX:2 * PIX]   # spare space in bank 2
            for kt in range(KT_C):
                nc.tensor.matmul(
                    mps, lhsT=wsum_sb[:, kt, 0, :], rhs=clip_sb[:, kt, 0, :],
                    start=(kt == 0), stop=(kt == KT_C - 1),
                    skip_group_check=True,
                )
            m16 = cvw.tile([1, 16, 16], BF16, tag="m16", name="m16")
            nc.vector.tensor_scalar(
                out=m16, in0=mps.rearrange("p (y x) -> p y x", y=16),
                scalar1=bsum_sb[0:1, :], scalar2=-1.0 / D,
                op0=AOP.add, op1=AOP.mult,
            )
            _upsample2x(nc, cvw, m16, m32n.rearrange("p (y x) -> p y x", y=32),
                        1, "m", "m", nc.gpsimd.tensor_scalar_mul)
            mu2 = cvw.tile([1, S], BF16, tag="mu2", name="mu2")
            nc.vector.tensor_mul(mu2, m32n, m32n)
            for t in range(2, KT_D):
                conv_tile(t)
                sq_mm(t - 2)
            q_early(0)
            sq_mm(KT_D - 2)
            q_early(1)
            sq_mm(KT_D - 1)
            q_early(2)
            q_early(3)

            # ---- squared-bilinear variance assembly (integer-coefficient
            # stencils; the /16 per stage folds into the final 1/(D*256)).
            # x-stage: M2x = 9*A_i + A_{i-1} + 6*B_{i-1} (planar-x [16,32])
            # DVE reads at most one PSUM operand per op: stage the summed
            # maps to SBUF first (single ACT copy)
            mm_sb = cvw.tile([1, 3, SCHUNK], BF16, tag="mmsb", name="mm_sb")
            nc.scalar.copy(out=mm_sb[:, 0, :], in_=mm_ps[:, 0, :])
            A = mm_sb[0:1, 0, 0:256].rearrange("p (y x) -> p y x", y=16)
            Bm = mm_sb[0:1, 0, 256:496].rearrange("p (y x) -> p y x", y=16)
            Cm = mm_sb[0:1, 1, 0:240].rearrange("p (y x) -> p y x", y=15)
            Dm = mm_sb[0:1, 1, 240:465].rearrange("p (y x) -> p y x", y=15)
            Dp = mm_sb[0:1, 2, 0:225].rearrange("p (y x) -> p y x", y=15)
            m2x = cvw.tile([1, 16, 32], BF16, tag="m2x", name="m2x")
            xm = cvw.tile([1, 15, 32], BF16, tag="xm", name="xm")
            tvar = cvw.tile([1, 16, 32], BF16, tag="tvar", name="tvar")
            ds = cvw.tile([1, 15, 15], BF16, tag="ds", name="ds")
            # M2x even/odd
            nc.vector.scalar_tensor_tensor(
                out=tvar[:, :, 1:16], in0=A[:, :, 1:16], scalar=9.0,
                in1=A[:, :, 0:15], op0=AOP.mult, op1=AOP.add)
            nc.vector.scalar_tensor_tensor(
                out=m2x[:, :, 1:16], in0=Bm[:, :, 0:15], scalar=6.0,
                in1=tvar[:, :, 1:16], op0=AOP.mult, op1=AOP.add)
            nc.vector.tensor_scalar_mul(m2x[:, :, 0:1], A[:, :, 0:1], 16.0)
            nc.vector.scalar_tensor_tensor(
                out=tvar[:, :, 16:31], in0=A[:, :, 0:15], scalar=9.0,
                in1=A[:, :, 1:16], op0=AOP.mult, op1=AOP.add)
            nc.vector.scalar_tensor_tensor(
                out=m2x[:, :, 16:31], in0=Bm[:, :, 0:15], scalar=6.0,
                in1=tvar[:, :, 16:31], op0=AOP.mult, op1=AOP.add)
            nc.vector.tensor_scalar_mul(m2x[:, :, 31:32], A[:, :, 15:16], 16.0)
            # Xm = 9*C_i + C_{i-1} + 3*(D+D')_{i-1}
            nc.scalar.copy(out=mm_sb[:, 1, :], in_=mm_ps[:, 1, :])
            nc.scalar.copy(out=mm_sb[:, 2, 0:PIX], in_=mm_ps[:, 2, 0:PIX])
            nc.vector.tensor_add(ds, Dm, Dp)
            nc.vector.scalar_tensor_tensor(
                out=tvar[:, 0:15, 1:16], in0=Cm[:, :, 1:16], scalar=9.0,
                in1=Cm[:, :, 0:15], op0=AOP.mult, op1=AOP.add)
            nc.vector.scalar_tensor_tensor(
                out=xm[:, :, 1:16], in0=ds, scalar=3.0,
                in1=tvar[:, 0:15, 1:16], op0=AOP.mult, op1=AOP.add)
            nc.vector.tensor_scalar_mul(xm[:, :, 0:1], Cm[:, :, 0:1], 16.0)
            nc.vector.scalar_tensor_tensor(
                out=tvar[:, 0:15, 16:31], in0=Cm[:, :, 0:15], scalar=9.0,
                in1=Cm[:, :, 1:16], op0=AOP.mult, op1=AOP.add)
            nc.vector.scalar_tensor_tensor(
                out=xm[:, :, 16:31], in0=ds, scalar=3.0,
                in1=tvar[:, 0:15, 16:31], op0=AOP.mult, op1=AOP.add)
            nc.vector.tensor_scalar_mul(xm[:, :, 31:32], Cm[:, :, 15:16], 16.0)
            # y-stage into var_row (interleaved y): M2f = 9*M2x_m + M2x_{m∓1}
            # + 6*Xm; then var = M2f/(D*256) - mu^2 in a second pass
            vr = var_row.rearrange("p (m two x) -> p m two x", two=2, x=32)
            vev = vr[:, :, 0, :]
            vod = vr[:, :, 1, :]
            nc.vector.scalar_tensor_tensor(
                out=tvar[:, 1:16, :], in0=m2x[:, 1:16, :], scalar=9.0,
                in1=m2x[:, 0:15, :], op0=AOP.mult, op1=AOP.add)
            nc.vector.scalar_tensor_tensor(
                out=vev[:, 1:16, :], in0=xm, scalar=6.0,
                in1=tvar[:, 1:16, :], op0=AOP.mult, op1=AOP.add)
            nc.vector.tensor_scalar_mul(vev[:, 0:1, :], m2x[:, 0:1, :], 16.0)
            nc.vector.scalar_tensor_tensor(
                out=tvar[:, 0:15, :], in0=m2x[:, 0:15, :], scalar=9.0,
                in1=m2x[:, 1:16, :], op0=AOP.mult, op1=AOP.add)
            nc.vector.scalar_tensor_tensor(
                out=vod[:, 0:15, :], in0=xm, scalar=6.0,
                in1=tvar[:, 0:15, :], op0=AOP.mult, op1=AOP.add)
            nc.vector.tensor_scalar_mul(vod[:, 15:16, :], m2x[:, 15:16, :], 16.0)
            # var = M2f/(D*256) - mu^2 ; rstd = 1/sqrt(var + eps)
            for ck in range(NCK):
                sl = slice(ck * SCHUNK, (ck + 1) * SCHUNK)
                nc.vector.scalar_tensor_tensor(
                    out=var_row[:, sl], in0=var_row[:, sl],
                    scalar=1.0 / (D * 256.0),
                    in1=mu2[:, sl], op0=AOP.mult, op1=AOP.subtract,
                )
        nc.scalar.activation(var_row, var_row, AFT.Sqrt, bias=eps1_col[0:1, :])
        rstd_bf = cvw.tile([1, S], BF16, tag="rstdbf", name="rstd_bf")
        with nc.allow_low_precision(reason="rstd applied to bf16 q anyway"):
            nc.vector.reciprocal(rstd_bf, var_row)
        nc.gpsimd.partition_broadcast(rstd_b, rstd_bf)
        # q97 row: -mu * rstd (bf16), shared across heads; duplicated into
        # a 2-row tile so each q head fills rows 96..97 with one DVE copy
        # (row 97 pairs with a zero k-column, any finite value works)
        nc.vector.tensor_mul(nmr_row, m32n, rstd_bf)
        nc.gpsimd.partition_broadcast(nmr2, nmr_row)

        cvw_cm.__exit__(None, None, None)
        otp = ctx.enter_context(tc.tile_pool(name="otp", bufs=1))
        oT_sb = otp.tile([HD, NH, S], BF16, tag="oT", name="oT_sb")
        # d-major repack of oT for the 6-deep out-projection
        oTd = otp.tile([P, KT_D, S], BF16, tag="oTd", name="oTd")
        att_cm = tc.tile_pool(name="pssc", bufs=2, space="PSUM")
        pssc = att_cm.__enter__()
        att2_cm = tc.tile_pool(name="psa", bufs=2, space="PSUM")
        psa = att2_cm.__enter__()

        # ================= attention =================
        def k_head_ic(h, dst, ic):
            # fp8 DoubleRow: hi x hi over kt pairs + cross-term corrections;
            # moving free is capped at 512 = 2x256, so 256-wide groups.
            # 98 output features: 96 k rows + k97 = x^T (wk @ wqsum) + 0-pad
            hsl = slice(h * HD1, (h + 1) * HD1)
            if True:
                pp = psq.tile([HD1, SCHUNK], F32, tag="ps", name=f"pp{h}_{ic}")
                for g in range(2):
                    gsl = slice(ic * SCHUNK + g * 256, ic * SCHUNK + (g + 1) * 256)
                    out = pp[:, g * 256:(g + 1) * 256]
                    for tp in range(KT_D // 2):
                        nc.tensor.matmul(
                            out, lhsT=wk_sb[:, 2 * tp:2 * tp + 2, 1, hsl],
                            rhs=xT_sb[:, 2 * tp:2 * tp + 2, 0, gsl],
                            start=(tp == 0), stop=False, perf_mode=DR,
                        )
                    for kt in range(KT_D):
                        nc.tensor.matmul(
                            out, lhsT=wk_sb[:, kt, 0:2, hsl],
                            rhs=xT_sb[:, kt, 0:2, gsl],
                            start=False, stop=(kt == KT_D - 1), perf_mode=DR,
                        )
                isl = slice(ic * SCHUNK, (ic + 1) * SCHUNK)
                if has_bk:
                    nc.vector.tensor_scalar(
                        out=dst[:, isl], in0=pp, scalar1=1.0 / WS,
                        scalar2=bk_sb[:, h:h + 1], op0=AOP.mult, op1=AOP.add)
                elif h < 4:
                    nc.scalar.mul(dst[:, isl], pp, 1.0 / WS)
                else:
                    nc.vector.tensor_scalar_mul(dst[:, isl], pp, 1.0 / WS)

        def scores_head(h, q_sb, k_sb, probs):
            for jt in range(NT_S):
                ps2 = pssc.tile([P, S], F32, tag="sc", name=f"ps{h}_{jt}")
                for ic in range(NCK):
                    isl = slice(ic * SCHUNK, (ic + 1) * SCHUNK)
                    nc.tensor.matmul(
                        ps2[:, isl], lhsT=k_sb[:, jt * P:(jt + 1) * P],
                        rhs=q_sb[:, isl], start=True, stop=True,
                    )
                nc.scalar.activation(probs[:, jt, :], ps2, AFT.Exp, scale=SCALE)

        def repack(h, ic):
            # oT head h (96 rows) -> d-major oTd rows h*96..h*96+95, which
            # spans at most two 128-row tiles; SBUF->SBUF DMA does the
            # partition remap off the engines. Issued per token-half right
            # after each AV copyback, on rotating queues, so the last
            # head's repack latency is mostly hidden.
            f0 = h * HD
            t0, r0 = f0 // P, f0 % P
            n0 = min(P - r0, HD)
            isl = slice(ic * SCHUNK, (ic + 1) * SCHUNK)
            if h >= NH - 2:
                return  # h6/h7 feed the out-projection head-major, no repack
            q0, q1 = (nc.scalar, nc.gpsimd) if ic == 0 else (nc.sync, nc.scalar)
            q0.dma_start(
                out=oTd[r0:r0 + n0, t0, isl], in_=oT_sb[0:n0, h, isl])
            if n0 < HD:
                q1.dma_start(
                    out=oTd[0:HD - n0, t0 + 1, isl], in_=oT_sb[n0:HD, h, isl])

        def av_head(h, probs):
            for ic in range(NCK):
                isl = slice(ic * SCHUNK, (ic + 1) * SCHUNK)
                po = psa.tile([HD + 1, SCHUNK], F32, tag="acc", name=f"po{h}_{ic}")
                for jt in range(NT_S):
                    nc.tensor.matmul(
                        po, lhsT=v_hsb[:, jt, h, :], rhs=probs[:, jt, isl],
                        start=(jt == 0), stop=(jt == NT_S - 1),
                    )
                zr = zp.tile([1, SCHUNK], F32, tag="zr", name=f"zr{h}_{ic}")
                nc.vector.reciprocal(zr, po[HD:HD + 1, :])
                zb = zp.tile([HD, SCHUNK], F32, tag="zb", name=f"zb{h}_{ic}")
                nc.gpsimd.partition_broadcast(zb, zr)
                nc.vector.tensor_mul(oT_sb[:, h, isl], po[0:HD, :], zb)
                repack(h, ic)

        def v_pair(h):
            # V for heads h, h+1 in token-partition layout, 2 jt per PSUM tile
            hsl2 = slice(h * HD, (h + 2) * HD)
            for g in range(4):
                pv = psa.tile([P, 2, 2 * HD], F32, tag="acc", name=f"pv{h}_{g}")
                for j in range(2):
                    jt = 2 * g + j
                    jsl = slice(jt * P, (jt + 1) * P)
                    for tp in range(KT_D // 2):
                        nc.tensor.matmul(
                            pv[:, j, :],
                            lhsT=xT_sb[:, 2 * tp:2 * tp + 2, 0, jsl],
                            rhs=wv_sb[:, 2 * tp:2 * tp + 2, 1, hsl2],
                            start=(tp == 0), stop=False, perf_mode=DR,
                        )
                    for kt in range(KT_D):
                        nc.tensor.matmul(
                            pv[:, j, :],
                            lhsT=xT_sb[:, kt, 0:2, jsl],
                            rhs=wv_sb[:, kt, 0:2, hsl2],
                            start=False, stop=(kt == KT_D - 1), perf_mode=DR,
                        )
                dst = v_hsb[:, 2 * g:2 * g + 2, h:h + 2, 0:HD]
                pv_r = pv.rearrange("p j (hh q) -> p j hh q", hh=2)
                if has_bv:
                    bv_s = bv_b[:, h * HD:(h + 2) * HD].rearrange(
                        "p (hh q) -> p hh q", hh=2)
                    for j in range(2):
                        nc.vector.scalar_tensor_tensor(
                            out=dst[:, j], in0=pv_r[:, j], scalar=1.0 / WS,
                            in1=bv_s, op0=AOP.mult, op1=AOP.add)
                else:
                    nc.vector.tensor_scalar_mul(dst, pv_r, 1.0 / WS)

        ks = {}
        probs = {}

        def k_(h):
            ks[h] = qk.tile([HD1, S], BF16, tag="k", name=f"k{h}")
            k_head_ic(h, ks[h], 0)
            k_head_ic(h, ks[h], 1)

        def sc(h):
            probs[h] = pr.tile([P, NT_S, S], BF16, tag="probs", name=f"probs{h}")
            scores_head(h, qs[h], ks[h], probs[h])

        def av(h):
            av_head(h, probs[h])

        # software pipeline: conv (above) -> q heads (16-grid, cheap) while
        # x/wk stream -> k/scores/AV interleaved; scores run ~2 ahead of AV
        # so softmax exp on ACT never blocks the tensor engine.
        q_late(0); q_late(1); q_late(2); q_late(3)
        ks[0] = qk.tile([HD1, S], BF16, tag="k", name="k0")
        ks[1] = qk.tile([HD1, S], BF16, tag="k", name="k1")
        k_head_ic(0, ks[0], 0)
        k_head_ic(1, ks[1], 0)
        k_head_ic(0, ks[0], 1)
        sc(0)
        k_head_ic(1, ks[1], 1)
        sc(1); q_head(4)
        k_(2); sc(2); v_pair(0); av(0); q_head(5)
        k_(3); sc(3); av(1); v_pair(2); q_head(6)
        k_(4); sc(4); av(2); q_head(7)
        k_(5); sc(5); av(3); v_pair(4)
        k_(6); sc(6); av(4)
        k_(7); sc(7)
        warm2 = wts.tile([1, 1], F32, tag="warm2", name="warm2")
        nc.scalar.activation(warm2, eps1_col, AFT.Sqrt)
        v_pair(6); av(5)
        # out-proj st0 kt 0..3 accumulate (from the now-idle psq pool) while
        # ACT drains exp(6/7); kt4 joins after av(6)'s repack, kt5 after
        # av(7)'s. (kt tile t is fed by heads floor(t*128/96)..)
        p2s = {}

        def op_mm(p2, st, nk, kt, start, stop, sgc=False):
            # kt 0..3 d-major; kt4 holds only h5's 64 rows; kt5 is replaced
            # by head-major h6+h7 terms read straight from oT_sb
            ssl = slice(st * P, (st + 1) * P)
            nsl = slice(nk * OCHUNK, (nk + 1) * OCHUNK)
            if kt == KT_D - 2:
                nc.tensor.matmul(
                    p2, lhsT=oTd[0:64, kt, ssl], rhs=wo_sb[0:64, kt, nsl],
                    start=start, stop=False, skip_group_check=sgc)
                nc.tensor.matmul(
                    p2, lhsT=oT_sb[:, NH - 2, ssl], rhs=wo6_sb[:, nsl],
                    start=False, stop=stop, skip_group_check=sgc)
            elif kt == KT_D - 1:
                nc.tensor.matmul(
                    p2, lhsT=oT_sb[:, NH - 1, ssl], rhs=wo7_sb[:, nsl],
                    start=start, stop=stop, skip_group_check=sgc)
            else:
                nc.tensor.matmul(
                    p2, lhsT=oTd[:, kt, ssl], rhs=wo_sb[:, kt, nsl],
                    start=start, stop=stop, skip_group_check=sgc)

        def pre_kts(st, kts):
            for nk in range(2):
                if (st, nk) not in p2s:
                    p2s[(st, nk)] = psq.tile(
                        [P, OCHUNK], F32, tag="ps", name=f"po2_{st}_{nk}")
                for kt in kts:
                    op_mm(p2s[(st, nk)], st, nk, kt,
                          kt == 0, False, sgc=True)

        pre_kts(0, range(4))
        av(6)
        pre_kts(0, [4])
        av(7)
        for nk in range(2):
            op_mm(p2s[(0, nk)], 0, nk, 5, False, True, sgc=True)

        # ================= out-projection + final LayerNorm =================
        # attention PSUM pools closed; out-proj uses a 6-buf pool so three
        # token tiles stay in flight while LN2 reads accumulators from PSUM
        att2_cm.__exit__(None, None, None)
        att_cm.__exit__(None, None, None)
        with tc.tile_pool(name="pso", bufs=6, space="PSUM") as pso:
            for st in range(NT_S):
                srcs = []
                st6 = stp.tile([P, 2, 6], F32, tag="st6", name=f"st6_{st}")
                for nk in range(2):
                    if (st, nk) in p2s:
                        p2 = p2s[(st, nk)]
                    else:
                        p2 = pso.tile([P, OCHUNK], F32, tag="po2", name=f"po2_{st}_{nk}")
                        for kt in range(KT_D):
                            op_mm(p2, st, nk, kt, kt == 0, kt == KT_D - 1)
                    if has_bo:
                        tb = op.tile([P, OCHUNK], F32, tag=f"tb{nk}", name=f"tb{st}_{nk}")
                        nc.vector.tensor_add(tb, p2, bo_b[:, nk * OCHUNK:(nk + 1) * OCHUNK])
                        srcs.append(tb)
                    else:
                        srcs.append(p2)
                    nc.vector.bn_stats(out=st6[:, nk, :], in_=srcs[nk])
                mv = stp.tile([P, 2], F32, tag="mv", name=f"mv{st}")
                nc.vector.bn_aggr(out=mv, in_=st6)
                stdc = stp.tile([P, 1], F32, tag="stdc", name=f"stdc{st}")
                nc.scalar.activation(stdc, mv[:, 1:2], AFT.Sqrt, bias=eps2_col)
                rstdc = stp.tile([P, 1], F32, tag="rstdc", name=f"rstdc{st}")
                nc.vector.reciprocal(rstdc, stdc)
                # normalize on ACT: out = src*rstd + (-mu*rstd), per-partition
                nmr = stp.tile([P, 1], F32, tag="nmr", name=f"nmr{st}")
                nc.vector.tensor_scalar(
                    out=nmr, in0=mv[:, 0:1], scalar1=rstdc, scalar2=-1.0,
                    op0=AOP.mult, op1=AOP.mult,
                )
                out_sb = op.tile([P, D], F32, tag="out", name=f"out_sb{st}")
                nc.scalar.activation(
                    out_sb[:, 0:OCHUNK], srcs[0], AFT.Identity,
                    bias=nmr, scale=rstdc)
                nc.vector.tensor_scalar(
                    out=out_sb[:, OCHUNK:D], in0=srcs[1], scalar1=mv[:, 0:1],
                    scalar2=rstdc, op0=AOP.subtract, op1=AOP.mult)
                if has_ln2:
                    nc.vector.tensor_mul(out_sb, out_sb, ln2w_b)
                    nc.vector.tensor_add(out_sb, out_sb, ln2b_b)
                orow = out_h[:][st * P:(st + 1) * P, :]
                if st == NT_S - 1:
                    for c0, qe in ((0, nc.sync), (192, nc.scalar),
                                   (384, nc.gpsimd), (576, nc.sync)):
                        qe.dma_start(out=orow[:, c0:c0 + 192],
                                     in_=out_sb[:, c0:c0 + 192])
                else:
                    nc.sync.dma_start(out=orow[:, 0:OCHUNK], in_=out_sb[:, 0:OCHUNK])
                    nc.scalar.dma_start(out=orow[:, OCHUNK:D], in_=out_sb[:, OCHUNK:D])

    nc.compile()
    return nc


def _get_graph(flags):
    if flags not in _CACHE:
        _CACHE[flags] = build_graph(flags)
    return _CACHE[flags]


def make_in_maps(**inputs):
    """Host-side prep: fold ln1 into wq, cast to bf16, transpose x."""
    import ml_dtypes

    bf = ml_dtypes.bfloat16
    f32 = np.float32
    x = np.asarray(inputs["x"], f32)
    clip = np.asarray(inputs["clip_features"], f32)
    conv_w = np.asarray(inputs["conv_w"], f32)
    conv_b = np.asarray(inputs["conv_b"], f32)
    ln1_w = np.asarray(inputs["ln1_w"], f32)
    ln1_b = np.asarray(inputs["ln1_b"], f32)
    wq = np.asarray(inputs["wq"], f32)
    bq = np.asarray(inputs["bq"], f32)
    wk = np.asarray(inputs["wk"], f32)
    bk = np.asarray(inputs["bk"], f32)
    wv = np.asarray(inputs["wv"], f32)
    bv = np.asarray(inputs["bv"], f32)
    wo = np.asarray(inputs["wo"], f32)
    bo = np.asarray(inputs["bo"], f32)
    ln2_w = np.asarray(inputs["ln2_w"], f32)
    ln2_b = np.asarray(inputs["ln2_b"], f32)

    wq_eff = ln1_w[:, None] * wq
    bq_eff = bq + ln1_b @ wq
    # q97/k97 mean-subtraction feature: wqsum[m] = sum_d wq_eff[d, m];
    # per head h, wk97_h = wk_h @ wqsum_h so that
    # k97 = x^T wk97 pairs with q97 = -mu*rstd in the score dot product.
    wqsum = wq_eff.sum(axis=0)                       # [D]
    wk_h = wk.reshape(D, NH, HD)
    ws_h = wqsum.reshape(NH, HD)
    wk97 = np.einsum("dhm,hm->dh", wk_h, ws_h)       # [D, NH]
    wk_ext = np.concatenate(
        [wk_h, wk97[:, :, None], np.zeros((D, NH, 1), f32)],
        axis=2).reshape(D, NH * HD1)

    flags = (
        bool(np.any(bq_eff)),
        bool(np.any(bk)),
        bool(np.any(bv)),
        bool(np.any(bo)),
        bool(np.any(ln2_w != 1.0) or np.any(ln2_b)),
    )

    e4 = ml_dtypes.float8_e4m3

    def hilo(a):  # activations: fp8 hi at index 0, residual lo at 1 (axis -2)
        hi = np.asarray(a, f32).astype(e4)
        lo = (np.asarray(a, f32) - hi.astype(f32)).astype(e4)
        return np.stack([hi, lo], axis=-2)

    def lohi_w(a):  # weights: scaled by WS; residual at 0, main at 1
        s = np.asarray(a, f32) * WS
        hi = s.astype(e4)
        lo = (s - hi.astype(f32)).astype(e4)
        return np.stack([lo, hi], axis=-2)

    def dev_kp(w):  # [K, M] -> [P, kt, M], k-tile-major
        kt = w.shape[0] // P
        return w.reshape(kt, P, w.shape[1]).transpose(1, 0, 2)

    # conv_w t-major: [P, t, kt, {dw,w}, m] with ch = kt*128+p, d = t*128+m
    cw_t = conv_w.reshape(KT_C, P, KT_D, P).transpose(1, 2, 0, 3)
    wsum8 = conv_w.sum(axis=1).reshape(KT_C, P).T.astype(e4)  # [P, kt]

    shared = {
        "conv_w": np.ascontiguousarray(lohi_w(cw_t)).reshape(P, -1),
        "wsum": np.ascontiguousarray(
            np.stack([wsum8, wsum8], axis=-1)).reshape(P, -1),
        "bsum": np.full((1, 1), conv_b.sum(), dtype=f32),
        "wq": np.ascontiguousarray(dev_kp(wq_eff).astype(bf)).reshape(P, -1),
        "wk": np.ascontiguousarray(lohi_w(dev_kp(wk_ext))).reshape(P, -1),
        "wv": np.ascontiguousarray(lohi_w(dev_kp(wv))).reshape(P, -1),
        "wo": np.ascontiguousarray(dev_kp(wo).astype(bf)).reshape(P, -1),
        "wo6": np.ascontiguousarray(wo[(NH - 2) * HD:(NH - 1) * HD, :].astype(bf)),
        "wo7": np.ascontiguousarray(wo[(NH - 1) * HD:, :].astype(bf)),
        "cb": np.ascontiguousarray(conv_b.reshape(KT_D, P).T, dtype=f32),
    }
    if flags[0]:
        shared["bq"] = np.ascontiguousarray(bq_eff.reshape(NH, HD).T, dtype=f32)
    if flags[1]:
        bk97 = np.einsum("hm,hm->h", bk.reshape(NH, HD), ws_h)
        bk_ext = np.concatenate(
            [bk.reshape(NH, HD), bk97[:, None],
             np.zeros((NH, 1), f32)], axis=1)              # [NH, HD1]
        shared["bk"] = np.ascontiguousarray(bk_ext.T, dtype=f32)
    if flags[2]:
        shared["bv"] = np.ascontiguousarray(bv[None, :], dtype=f32)
    if flags[3]:
        shared["bo"] = np.ascontiguousarray(bo[None, :], dtype=f32)
    if flags[4]:
        shared["ln2w"] = np.ascontiguousarray(ln2_w[None, :], dtype=f32)
        shared["ln2b"] = np.ascontiguousarray(ln2_b[None, :], dtype=f32)

    in_maps = []
    for b in range(B):
        m = dict(shared)
        m["xT"] = np.ascontiguousarray(
            hilo(dev_kp(x[b].reshape(S, D).T))).reshape(P, -1)
        m["clip"] = np.ascontiguousarray(
            hilo(dev_kp(clip[b].reshape(CH, PIX)))).reshape(P, -1)
        in_maps.append(m)
    return flags, in_maps


def kernel(**inputs):
    global LAST_RESULT
    flags, in_maps = make_in_maps(**inputs)
    nc = _get_graph(flags)
    res = run_bass_kernel_spmd(nc, in_maps, core_ids=list(range(B)), trace=_TRACE)
    LAST_RESULT = res
    out = np.stack([r["out"] for r in res.results], axis=0)
    # un-permute x-planar token order: dram row rho holds raster token
    # (rho//32)*32 + 2*(rho%16) + (rho%32)//16
    rho = np.arange(S)
    raster = (rho // 32) * 32 + 2 * (rho % 16) + (rho % 32) // 16
    inv = np.empty(S, np.int64)
    inv[raster] = rho
    out = out[:, inv, :]
    return np.ascontiguousarray(out.reshape(B, HH, WW, D), dtype=np.float32)
